# revision 6
# baseline (speedup 1.0000x reference)
"""Trainium2 Bass kernel for nn_Attention (B=8, N=1025, C=768, H=12).

Strategy: pure data-parallel over batch — each of the 8 NeuronCores runs the
full attention block for one batch element.  All device-side tensors live in
"transposed" [channel, token] layout so no on-device transposes are needed:

  qkvT[ch, tok] = wqkvT.T @ xT           (q,k parts; ch on partitions)
  RoPE via a signed-permutation matmul (rotate_half = S @ q) + DVE elementwise
  v[tok, ch]    = xT.T @ wvT             (tok on partitions, per-head 65-col
                                          blocks with a ones column appended)
  ST[k, q]      = kT.T @ qT              (per head; head PAIRS row-packed on
                                          the 128x128 PE array, K=64 each)
  P = exp(ST * scale)                    (ScalarE, no max subtraction --
                                          logits have std ~2, max ~11, safe)
  O_aug[65, q]  = v_aug.T @ P            (row 64 = softmax denominator via the
                                          ones column; accumulated in PSUM)
  normalize with 1/denom broadcast across partitions via a small DRAM-staged
  stride-0 DMA, then yT = wprojT.T @ oT.

Matmuls run as float32r (TF32-like 11-bit-mantissa fp32, full PE rate for
free dims >= 256).  fp32r constraints handled here: output free dim must be
even (tokens padded 1025->1026, the pad token's V row and ones column are
zeroed so softmax is unaffected), every matmul input must be produced
rounded-to-fp32r (DMA-loaded operands declared f32r end-to-end with host
pre-rounding; compute-produced operands written with f32r-typed output APs;
memset can't write f32r so the ones constants are DMA-loaded).
"""

import numpy as np

import concourse.bass as bass
import concourse.bacc as bacc
import concourse.tile as tile
from concourse import mybir
from concourse.bass_utils import run_bass_kernel_spmd

B, N, C, H, HD = 8, 1025, 768, 12, 64
NP = 1026                # padded token count (fp32r needs even free dims)
SCALE = HD ** -0.5
KT = C // 128            # 6 contraction tiles over channels
NT = (NP + 127) // 128   # 9 token tiles (last holds 2 tokens: 1 real + 1 pad)
TAILW = NP - 128 * (NT - 1)
QSLICES = [(0, 512), (512, 512), (1024, TAILW)]
F32 = mybir.dt.float32
F32R = mybir.dt.float32r
AFT = mybir.ActivationFunctionType


def _r(ap):
    return ap.bitcast(F32R)


def _bcast_row(row_ap, parts):
    """AP reading a [1, W] DRAM row replicated across `parts` partitions."""
    return bass.AP(
        tensor=row_ap.tensor,
        offset=row_ap.offset,
        ap=[[0, parts]] + list(row_ap.ap[1:]),
    )


def _emit(nc, tc, d, dbg=False):
    from contextlib import ExitStack

    with ExitStack() as ctx:
        const = ctx.enter_context(tc.tile_pool(name="const", bufs=1))
        s2t = const.tile([128, 128], F32R, name="s2t_sb")
        nc.sync.dma_start(out=s2t, in_=d["s2t"])
        cost2 = const.tile([128, NP], F32, name="cost2_sb")
        nc.sync.dma_start(out=cost2, in_=d["cost2"])
        sint2 = const.tile([128, NP], F32, name="sint2_sb")
        nc.sync.dma_start(out=sint2, in_=d["sint2"])
        qkvbt = const.tile([128, 12], F32, name="qkvbt_sb")
        nc.sync.dma_start(out=qkvbt, in_=d["qkvbt"])
        projbt = const.tile([128, 6], F32, name="projbt_sb")
        nc.sync.dma_start(out=projbt, in_=d["projbt"])
        vb = const.tile([1, C], F32R, name="vb_sb")
        nc.sync.dma_start(out=vb, in_=d["vb"])
        ones1 = const.tile([1, 128], F32R, name="ones1")
        nc.sync.dma_start(out=ones1, in_=d["ones1"])

        qk_pool = ctx.enter_context(tc.tile_pool(name="qkp", bufs=1))
        qk = [qk_pool.tile([128, NP], F32, name=f"qk{m}") for m in range(12)]
        v_pool = ctx.enter_context(tc.tile_pool(name="vp", bufs=1))
        vsb = [v_pool.tile([128, 12, 65], F32R, name=f"v{t}") for t in range(NT)]

        # ---------------- phase 1: qkv projection + RoPE -------------------
        with tc.tile_pool(name="xw", bufs=1) as xw, \
             tc.tile_pool(name="ps1", bufs=4, space="PSUM") as ps1, \
             tc.tile_pool(name="tmp1", bufs=4) as tmp1:
            xt = [xw.tile([128, NP], F32R, name=f"xt{kk}") for kk in range(KT)]
            wq = [xw.tile([128, 3 * C], F32R, name=f"wq{kk}") for kk in range(KT)]
            for kk in range(KT):
                nc.sync.dma_start(out=wq[kk], in_=d["wqkvt"][kk * 128:(kk + 1) * 128, :])
                nc.sync.dma_start(out=xt[kk], in_=d["xt"][kk * 128:(kk + 1) * 128, :])

            # qT / kT in [ch, tok] layout (12 head-pair tiles of 128 channels)
            for m in range(12):
                for (q0, w) in QSLICES:
                    ps = ps1.tile([128, 512], F32, name="mmps")[:, :w]
                    for kk in range(KT):
                        nc.tensor.matmul(
                            ps,
                            wq[kk][:, m * 128:(m + 1) * 128],
                            xt[kk][:, q0:q0 + w],
                            start=(kk == 0), stop=(kk == KT - 1),
                        )
                    nc.scalar.activation(
                        _r(qk[m][:, q0:q0 + w]), ps, AFT.Identity,
                        bias=qkvbt[:, m:m + 1], scale=1.0,
                    )

            # v in [tok, ch] layout, blocked per head as [tok, 12, 65] with a
            # trailing ones column (softmax denominator rides the AV matmul).
            for t in range(NT):
                tw = 128 if t < NT - 1 else TAILW
                for vj, (v0, wv) in enumerate([(0, 512), (512, 256)]):
                    ps = ps1.tile([128, 512], F32, name="mmps")[:tw, :wv]
                    nc.tensor.matmul(ps, ones1[:1, :tw], vb[:1, v0:v0 + wv],
                                     start=True, stop=False)
                    for kk in range(KT):
                        nc.tensor.matmul(
                            ps,
                            xt[kk][:, t * 128:t * 128 + tw],
                            wq[kk][:, 2 * C + v0:2 * C + v0 + wv],
                            start=False, stop=(kk == KT - 1),
                        )
                    nc.vector.tensor_copy(
                        vsb[t][:tw, vj * 8:vj * 8 + wv // 64, 0:64],
                        ps.rearrange("p (h dd) -> p h dd", dd=64),
                    )
                # ones column for real tokens; the tail tile additionally gets
                # its pad-token row fully zeroed (keeps denominators and AV
                # contributions exact even with nonzero biases).
                if t < NT - 1:
                    nc.sync.dma_start(out=vsb[t][:tw, :, 64:65], in_=d["vones"][:tw])
                else:
                    nc.sync.dma_start(out=vsb[t][0:tw - 1, :, 64:65],
                                      in_=d["vones"][:tw - 1])
                    nc.sync.dma_start(out=vsb[t][tw - 1:tw, :, :], in_=d["vzrow"])

            # RoPE (in place on qk): rope = qk*cos + (S @ qk)*sin.
            # cls + pad tokens handled by host-side cos=1 / sin=0 columns.
            for m in range(12):
                for (q0, w) in QSLICES:
                    rps = ps1.tile([128, 512], F32, name="ropeps", bufs=2)[:, :w]
                    nc.tensor.matmul(rps, s2t, _r(qk[m][:, q0:q0 + w]),
                                     start=True, stop=True)
                    a_t = tmp1.tile([128, 512], F32, name="ropea")[:, :w]
                    nc.gpsimd.tensor_mul(a_t, qk[m][:, q0:q0 + w], cost2[:, q0:q0 + w])
                    b_t = tmp1.tile([128, 512], F32, name="ropeb")[:, :w]
                    nc.vector.tensor_mul(b_t, rps, sint2[:, q0:q0 + w])
                    nc.vector.tensor_add(_r(qk[m][:, q0:q0 + w]), a_t, b_t)

        if dbg:
            for m in range(12):
                nc.sync.dma_start(out=d["dbg_qk"][m], in_=qk[m])
            for t in range(NT):
                nc.sync.dma_start(out=d["dbg_v"][t], in_=vsb[t].bitcast(F32))

        # ---------------- phase 2: attention -------------------------------
        att = ctx.enter_context(tc.tile_pool(name="att", bufs=1))
        projrhs = [att.tile([128, NP], F32R, name=f"prhs{hp}") for hp in range(6)]
        wproj = [att.tile([128, C], F32R, name=f"wp{kk}") for kk in range(KT)]
        for kk in range(KT):
            nc.sync.dma_start(out=wproj[kk], in_=d["wprojt"][kk * 128:(kk + 1) * 128, :])

        with tc.tile_pool(name="ppool", bufs=3) as ppool, \
             tc.tile_pool(name="opool", bufs=2, space="PSUM") as opool, \
             tc.tile_pool(name="spool", bufs=2, space="PSUM") as spool, \
             tc.tile_pool(name="dstage", bufs=2, space="DRAM") as dstage, \
             tc.tile_pool(name="obuf", bufs=2) as obuf:
            for hp in range(6):
                oraw_e = obuf.tile([65, NP], F32, name="oraw_e")
                oraw_o = obuf.tile([65, NP], F32, name="oraw_o")
                for (q0, w) in QSLICES:
                    o_e = opool.tile([65, 512], F32, name="o_e")[:, :w]
                    o_o = opool.tile([65, 512], F32, name="o_o")[:, :w]
                    for kt in range(NT):
                        kw = 128 if kt < NT - 1 else TAILW
                        k0 = kt * 128
                        s_e = spool.tile([128, 512], F32, name="s_e")[:kw, :w]
                        s_o = spool.tile([128, 512], F32, name="s_o")[:kw, :w]
                        # head pair row-packed: even head on PE rows 0-63,
                        # odd head on rows 64-127 (auto tile_position).
                        nc.tensor.matmul(s_e, _r(qk[6 + hp][0:64, k0:k0 + kw]),
                                         _r(qk[hp][0:64, q0:q0 + w]),
                                         start=True, stop=True)
                        nc.tensor.matmul(s_o, _r(qk[6 + hp][64:128, k0:k0 + kw]),
                                         _r(qk[hp][64:128, q0:q0 + w]),
                                         start=True, stop=True)
                        p_e = ppool.tile([128, 512], F32R, name="p_e")[:kw, :w]
                        p_o = ppool.tile([128, 512], F32R, name="p_o")[:kw, :w]
                        nc.scalar.activation(p_e, s_e, AFT.Exp, bias=0.0, scale=SCALE)
                        nc.scalar.activation(p_o, s_o, AFT.Exp, bias=0.0, scale=SCALE)
                        nc.tensor.matmul(o_e, vsb[kt][:kw, 2 * hp, :], p_e,
                                         start=(kt == 0), stop=(kt == NT - 1))
                        nc.tensor.matmul(o_o, vsb[kt][:kw, 2 * hp + 1, :], p_o,
                                         start=(kt == 0), stop=(kt == NT - 1))
                    nc.vector.tensor_copy(oraw_e[:, q0:q0 + w], o_e)
                    nc.vector.tensor_copy(oraw_o[:, q0:q0 + w], o_o)

                # denominators (row 64) -> reciprocal -> broadcast over the 64
                # head dims via DRAM-staged stride-0 DMA -> normalize.
                if dbg and hp == 0:
                    nc.sync.dma_start(out=d["dbg_oraw"][0], in_=oraw_e)
                    nc.sync.dma_start(out=d["dbg_oraw"][1], in_=oraw_o)
                # denominators (row 64) -> move to partitions 0-1 (the custom
                # DVE reciprocal only operates at partition base 0) -> one
                # batched reciprocal -> DRAM-staged stride-0 broadcast.
                den = obuf.tile([2, NP], F32, name="den")
                nc.sync.dma_start(out=den[0:1, :], in_=oraw_e[64:65, :])
                nc.sync.dma_start(out=den[1:2, :], in_=oraw_o[64:65, :])
                nc.vector.reciprocal_approx_fast(den, den)
                dtmp = dstage.tile([2, NP], F32, name="dtmp")
                nc.sync.dma_start(out=dtmp[:, :], in_=den)
                bc_e = obuf.tile([64, NP], F32, name="bc_e")
                bc_o = obuf.tile([64, NP], F32, name="bc_o")
                nc.gpsimd.dma_start(out=bc_e, in_=_bcast_row(dtmp[0:1, :], 64))
                nc.gpsimd.dma_start(out=bc_o, in_=_bcast_row(dtmp[1:2, :], 64))
                if dbg and hp == 0:
                    nc.sync.dma_start(out=d["dbg_norm"][0][:65], in_=oraw_e)
                    nc.sync.dma_start(out=d["dbg_norm"][1][:64], in_=bc_e)
                    nc.sync.dma_start(out=d["dbg_norm"][2][:64], in_=bc_o)
                nc.vector.tensor_mul(projrhs[hp][0:64, :], oraw_e[0:64, :], bc_e)
                onorm_o = obuf.tile([64, NP], F32R, name="onorm_o")
                nc.vector.tensor_mul(onorm_o, oraw_o[0:64, :], bc_o)
                if dbg and hp == 0:
                    nc.sync.dma_start(out=d["dbg_norm"][3][:64], in_=onorm_o.bitcast(F32))
                # odd head lives on partitions 64-127 of the proj rhs tile;
                # DMA is the only engine that can shift partition ranges.
                nc.sync.dma_start(out=projrhs[hp][64:128, :], in_=onorm_o)

        if dbg:
            for hp in range(6):
                nc.sync.dma_start(out=d["dbg_prhs"][hp], in_=projrhs[hp].bitcast(F32))

        # ---------------- phase 3: output projection -----------------------
        with tc.tile_pool(name="ps3", bufs=4, space="PSUM") as ps3, \
             tc.tile_pool(name="ybuf", bufs=3) as ybuf:
            for m in range(6):
                for (q0, w) in QSLICES:
                    ps = ps3.tile([128, 512], F32, name="yps")[:, :w]
                    for kk in range(KT):
                        nc.tensor.matmul(
                            ps,
                            wproj[kk][:, m * 128:(m + 1) * 128],
                            projrhs[kk][:, q0:q0 + w],
                            start=(kk == 0), stop=(kk == KT - 1),
                        )
                    yt = ybuf.tile([128, 512], F32, name="yt")[:, :w]
                    nc.scalar.activation(yt, ps, AFT.Identity,
                                         bias=projbt[:, m:m + 1], scale=1.0)
                    nc.sync.dma_start(out=d["out"][m * 128:(m + 1) * 128, q0:q0 + w],
                                      in_=yt)


_NC = None


def build_nc(dbg=False):
    global _NC
    if _NC is None or dbg:
        nc = bacc.Bacc("TRN2", target_bir_lowering=False, debug=False)
        d = {
            "xt": nc.dram_tensor("xt", [C, NP], F32R, kind="ExternalInput").ap(),
            "wqkvt": nc.dram_tensor("wqkvt", [C, 3 * C], F32R, kind="ExternalInput").ap(),
            "wprojt": nc.dram_tensor("wprojt", [C, C], F32R, kind="ExternalInput").ap(),
            "s2t": nc.dram_tensor("s2t", [128, 128], F32R, kind="ExternalInput").ap(),
            "cost2": nc.dram_tensor("cost2", [128, NP], F32, kind="ExternalInput").ap(),
            "sint2": nc.dram_tensor("sint2", [128, NP], F32, kind="ExternalInput").ap(),
            "qkvbt": nc.dram_tensor("qkvbt", [128, 12], F32, kind="ExternalInput").ap(),
            "projbt": nc.dram_tensor("projbt", [128, 6], F32, kind="ExternalInput").ap(),
            "vb": nc.dram_tensor("vb", [1, C], F32R, kind="ExternalInput").ap(),
            "ones1": nc.dram_tensor("ones1", [1, 128], F32R, kind="ExternalInput").ap(),
            "vones": nc.dram_tensor("vones", [128, 12, 1], F32R, kind="ExternalInput").ap(),
            "vzrow": nc.dram_tensor("vzrow", [1, 12, 65], F32R, kind="ExternalInput").ap(),
            "out": nc.dram_tensor("out", [C, NP], F32, kind="ExternalOutput").ap(),
        }
        if dbg:
            d["dbg_qk"] = nc.dram_tensor("dbg_qk", [12, 128, NP], F32, kind="ExternalOutput").ap()
            d["dbg_v"] = nc.dram_tensor("dbg_v", [NT, 128, 12, 65], F32, kind="ExternalOutput").ap()
            d["dbg_oraw"] = nc.dram_tensor("dbg_oraw", [2, 65, NP], F32, kind="ExternalOutput").ap()
            d["dbg_prhs"] = nc.dram_tensor("dbg_prhs", [6, 128, NP], F32, kind="ExternalOutput").ap()
            d["dbg_norm"] = nc.dram_tensor("dbg_norm", [4, 65, NP], F32, kind="ExternalOutput").ap()
        with tile.TileContext(nc) as tc:
            _emit(nc, tc, d, dbg=dbg)
        nc.compile()
        if dbg:
            return nc
        _NC = nc
    return _NC


def _round_fp32r(a):
    """Round fp32 array to the fp32r grid (11-bit mantissa, RNE)."""
    u = np.ascontiguousarray(a, dtype=np.float32).view(np.uint32).copy()
    u += 0x7FF + ((u >> 12) & 1)
    u &= 0xFFFFF000
    return u.view(np.float32)


def make_in_maps(inputs):
    x = np.ascontiguousarray(np.asarray(inputs["x"], dtype=np.float32))
    sin = np.asarray(inputs["sin"], dtype=np.float32)
    cos = np.asarray(inputs["cos"], dtype=np.float32)
    qkv_w = np.asarray(inputs["qkv_w"], dtype=np.float32)
    qkv_b = np.asarray(inputs["qkv_b"], dtype=np.float32)
    proj_w = np.asarray(inputs["proj_w"], dtype=np.float32)
    proj_b = np.asarray(inputs["proj_b"], dtype=np.float32)

    # rotate_half as a signed permutation: rot = S64 @ q (per 64-dim head).
    s64 = np.zeros((64, 64), dtype=np.float32)
    for dd in range(32):
        s64[dd, dd + 32] = -1.0
        s64[dd + 32, dd] = 1.0
    s2t = np.zeros((128, 128), dtype=np.float32)
    s2t[:64, :64] = s64.T
    s2t[64:, 64:] = s64.T

    # [128, NP] sin/cos in [dim, token] layout, doubled for the head pair in
    # each 128-channel tile; col 0 (cls) and col 1025 (pad) get cos=1, sin=0.
    cost2 = np.ones((128, NP), dtype=np.float32)
    sint2 = np.zeros((128, NP), dtype=np.float32)
    cost2[:64, 1:N] = cos.T
    cost2[64:, 1:N] = cos.T
    sint2[:64, 1:N] = sin.T
    sint2[64:, 1:N] = sin.T

    shared = {
        "wqkvt": _round_fp32r(qkv_w.T),
        "wprojt": _round_fp32r(proj_w.T),
        "s2t": s2t,
        "cost2": cost2,
        "sint2": sint2,
        "qkvbt": np.ascontiguousarray(qkv_b[:2 * C].reshape(12, 128).T),
        "projbt": np.ascontiguousarray(proj_b.reshape(6, 128).T),
        "vb": _round_fp32r(qkv_b[2 * C:].reshape(1, C)),
        "ones1": np.ones((1, 128), dtype=np.float32),
        "vones": np.ones((128, 12, 1), dtype=np.float32),
        "vzrow": np.zeros((1, 12, 65), dtype=np.float32),
    }
    xp = np.zeros((C, NP), dtype=np.float32)
    maps = []
    for b in range(B):
        xp[:, :N] = x[b].T
        maps.append(dict(shared, xt=_round_fp32r(xp)))
    return maps


def kernel(**inputs) -> np.ndarray:
    nc = build_nc()
    in_maps = make_in_maps(inputs)
    res = run_bass_kernel_spmd(nc, in_maps, core_ids=list(range(B)))
    return np.stack([res.results[b]["out"][:, :N].T for b in range(B)]).astype(np.float32)


# revision 7
# speedup vs baseline: 1.1720x; 1.1720x over previous
"""Trainium2 Bass kernel for nn_Attention (B=8, N=1025, C=768, H=12).

Strategy: pure data-parallel over batch — each of the 8 NeuronCores runs the
full attention block for one batch element.  All device-side tensors live in
"transposed" [channel, token] layout so no on-device transposes are needed:

  qkvT[ch, tok] = wqkvT.T @ xT           (q,k parts; ch on partitions)
  RoPE via a signed-permutation matmul (rotate_half = S @ q) + DVE elementwise
  v[tok, ch]    = xT.T @ wvT             (tok on partitions, per-head 65-col
                                          blocks with a ones column appended)
  ST[k, q]      = kT.T @ qT              (per head; head PAIRS row-packed on
                                          the 128x128 PE array, K=64 each)
  P = exp(ST * scale)                    (ScalarE, no max subtraction --
                                          logits have std ~2, max ~11, safe)
  O_aug[65, q]  = v_aug.T @ P            (row 64 = softmax denominator via the
                                          ones column; accumulated in PSUM)
  normalize with 1/denom broadcast across partitions via a small DRAM-staged
  stride-0 DMA, then yT = wprojT.T @ oT.

Matmuls run as float32r (TF32-like 11-bit-mantissa fp32, full PE rate for
free dims >= 256).  fp32r constraints handled here: output free dim must be
even (tokens padded 1025->1026, the pad token's V row and ones column are
zeroed so softmax is unaffected), every matmul input must be produced
rounded-to-fp32r (DMA-loaded operands declared f32r end-to-end with host
pre-rounding; compute-produced operands written with f32r-typed output APs;
memset can't write f32r so the ones constants are DMA-loaded).
"""

import numpy as np

import concourse.bass as bass
import concourse.bacc as bacc
import concourse.tile as tile
from concourse import mybir
from concourse.bass_utils import run_bass_kernel_spmd

B, N, C, H, HD = 8, 1025, 768, 12, 64
NP = 1026                # padded token count (fp32r needs even free dims)
SCALE = HD ** -0.5
KT = C // 128            # 6 contraction tiles over channels
NT = (NP + 127) // 128   # 9 token tiles (last holds 2 tokens: 1 real + 1 pad)
TAILW = NP - 128 * (NT - 1)
QSLICES = [(0, 342), (342, 342), (684, 342)]
F32 = mybir.dt.float32
F32R = mybir.dt.float32r
BF16 = mybir.dt.bfloat16
AFT = mybir.ActivationFunctionType


def _r(ap):
    return ap.bitcast(F32R)


def _bcast_row(row_ap, parts):
    """AP reading a [1, W] DRAM row replicated across `parts` partitions."""
    return bass.AP(
        tensor=row_ap.tensor,
        offset=row_ap.offset,
        ap=[[0, parts]] + list(row_ap.ap[1:]),
    )


def _emit(nc, tc, d, dbg=False):
    from contextlib import ExitStack

    with ExitStack() as ctx:
        const = ctx.enter_context(tc.tile_pool(name="const", bufs=1))
        s2t = const.tile([128, 128], F32R, name="s2t_sb")
        nc.sync.dma_start(out=s2t, in_=d["s2t"])
        cost2 = const.tile([128, NP], F32, name="cost2_sb")
        nc.sync.dma_start(out=cost2, in_=d["cost2"])
        sint2 = const.tile([128, NP], F32, name="sint2_sb")
        nc.sync.dma_start(out=sint2, in_=d["sint2"])
        qkvbt = const.tile([128, 12], F32, name="qkvbt_sb")
        nc.sync.dma_start(out=qkvbt, in_=d["qkvbt"])
        projbt = const.tile([128, 6], F32, name="projbt_sb")
        nc.sync.dma_start(out=projbt, in_=d["projbt"])
        vb = const.tile([1, C], F32R, name="vb_sb")
        nc.sync.dma_start(out=vb, in_=d["vb"])
        ones1 = const.tile([1, 128], F32R, name="ones1")
        nc.sync.dma_start(out=ones1, in_=d["ones1"])

        qk_pool = ctx.enter_context(tc.tile_pool(name="qkp", bufs=1))
        qkb = [qk_pool.tile([128, NP], BF16, name=f"qkb{m}") for m in range(12)]
        v_pool = ctx.enter_context(tc.tile_pool(name="vp", bufs=1))
        vsb = [v_pool.tile([128, 12, 65], BF16, name=f"v{t}") for t in range(NT)]

        # ---------------- phase 1: qkv projection + RoPE -------------------
        with tc.tile_pool(name="xw", bufs=1) as xw, \
             tc.tile_pool(name="ps1", bufs=4, space="PSUM") as ps1, \
             tc.tile_pool(name="tmp1", bufs=4) as tmp1:
            xt = [xw.tile([128, NP], F32R, name=f"xt{kk}") for kk in range(KT)]
            qk = [xw.tile([128, NP], F32, name=f"qkr{m}") for m in range(12)]
            wq = [xw.tile([128, 3 * C], F32R, name=f"wq{kk}") for kk in range(KT)]
            for kk in range(KT):
                nc.sync.dma_start(out=wq[kk], in_=d["wqkvt"][kk * 128:(kk + 1) * 128, :])
                nc.sync.dma_start(out=xt[kk], in_=d["xt"][kk * 128:(kk + 1) * 128, :])

            # qT / kT in [ch, tok] layout (12 head-pair tiles of 128 channels)
            for m in range(12):
                for (q0, w) in QSLICES:
                    ps = ps1.tile([128, 512], F32, name="mmps")[:, :w]
                    for kk in range(KT):
                        nc.tensor.matmul(
                            ps,
                            wq[kk][:, m * 128:(m + 1) * 128],
                            xt[kk][:, q0:q0 + w],
                            start=(kk == 0), stop=(kk == KT - 1),
                        )
                    nc.scalar.activation(
                        _r(qk[m][:, q0:q0 + w]), ps, AFT.Identity,
                        bias=qkvbt[:, m:m + 1], scale=1.0,
                    )

            # v in [tok, ch] layout, blocked per head as [tok, 12, 65] with a
            # trailing ones column (softmax denominator rides the AV matmul).
            for t in range(NT):
                tw = 128 if t < NT - 1 else TAILW
                for vj, (v0, wv) in enumerate([(0, 512), (512, 256)]):
                    ps = ps1.tile([128, 512], F32, name="mmps")[:tw, :wv]
                    nc.tensor.matmul(ps, ones1[:1, :tw], vb[:1, v0:v0 + wv],
                                     start=True, stop=False)
                    for kk in range(KT):
                        nc.tensor.matmul(
                            ps,
                            xt[kk][:, t * 128:t * 128 + tw],
                            wq[kk][:, 2 * C + v0:2 * C + v0 + wv],
                            start=False, stop=(kk == KT - 1),
                        )
                    nc.vector.tensor_copy(
                        vsb[t][:tw, vj * 8:vj * 8 + wv // 64, 0:64],
                        ps.rearrange("p (h dd) -> p h dd", dd=64),
                    )
                # ones column for real tokens; the tail tile additionally gets
                # its pad-token row fully zeroed (keeps denominators and AV
                # contributions exact even with nonzero biases).
                if t < NT - 1:
                    nc.sync.dma_start(out=vsb[t][:tw, :, 64:65], in_=d["vones"][:tw])
                else:
                    nc.sync.dma_start(out=vsb[t][0:tw - 1, :, 64:65],
                                      in_=d["vones"][:tw - 1])
                    nc.sync.dma_start(out=vsb[t][tw - 1:tw, :, :], in_=d["vzrow"])

            # RoPE (in place on qk): rope = qk*cos + (S @ qk)*sin.
            # cls + pad tokens handled by host-side cos=1 / sin=0 columns.
            for m in range(12):
                for (q0, w) in QSLICES:
                    rps = ps1.tile([128, 512], F32, name="ropeps", bufs=2)[:, :w]
                    nc.tensor.matmul(rps, s2t, _r(qk[m][:, q0:q0 + w]),
                                     start=True, stop=True)
                    a_t = tmp1.tile([128, 512], F32, name="ropea")[:, :w]
                    nc.gpsimd.tensor_mul(a_t, qk[m][:, q0:q0 + w], cost2[:, q0:q0 + w])
                    b_t = tmp1.tile([128, 512], F32, name="ropeb")[:, :w]
                    nc.vector.tensor_mul(b_t, rps, sint2[:, q0:q0 + w])
                    nc.vector.tensor_add(qkb[m][:, q0:q0 + w], a_t, b_t)

        if dbg:
            pass

        # ---------------- phase 2: attention -------------------------------
        att = ctx.enter_context(tc.tile_pool(name="att", bufs=1))
        projrhs = [att.tile([128, NP], F32R, name=f"prhs{hp}") for hp in range(6)]
        wproj = [att.tile([128, C], F32R, name=f"wp{kk}") for kk in range(KT)]
        for kk in range(KT):
            nc.sync.dma_start(out=wproj[kk], in_=d["wprojt"][kk * 128:(kk + 1) * 128, :])

        with tc.tile_pool(name="ppool", bufs=3) as ppool, \
             tc.tile_pool(name="opool", bufs=2, space="PSUM") as opool, \
             tc.tile_pool(name="spool", bufs=2, space="PSUM") as spool, \
             tc.tile_pool(name="dstage", bufs=2, space="DRAM") as dstage, \
             tc.tile_pool(name="obuf", bufs=2) as obuf:
            for hp in range(6):
                oraw_e = obuf.tile([65, NP], F32, name="oraw_e")
                oraw_o = obuf.tile([65, NP], F32, name="oraw_o")
                for (q0, w) in QSLICES:
                    o_e = opool.tile([65, 512], F32, name="o_e")[:, :w]
                    o_o = opool.tile([65, 512], F32, name="o_o")[:, :w]
                    for kt in range(NT):
                        kw = 128 if kt < NT - 1 else TAILW
                        k0 = kt * 128
                        s_e = spool.tile([128, 512], F32, name="s_e")[:kw, :w]
                        s_o = spool.tile([128, 512], F32, name="s_o")[:kw, :w]
                        # head pair row-packed: even head on PE rows 0-63,
                        # odd head on rows 64-127 (auto tile_position).
                        nc.tensor.matmul(s_e, qkb[6 + hp][0:64, k0:k0 + kw],
                                         qkb[hp][0:64, q0:q0 + w],
                                         start=True, stop=True)
                        nc.tensor.matmul(s_o, qkb[6 + hp][64:128, k0:k0 + kw],
                                         qkb[hp][64:128, q0:q0 + w],
                                         start=True, stop=True)
                        p_e = ppool.tile([128, 512], BF16, name="p_e")[:kw, :w]
                        p_o = ppool.tile([128, 512], BF16, name="p_o")[:kw, :w]
                        nc.scalar.activation(p_e, s_e, AFT.Exp, bias=0.0, scale=SCALE)
                        nc.scalar.activation(p_o, s_o, AFT.Exp, bias=0.0, scale=SCALE)
                        nc.tensor.matmul(o_e, vsb[kt][:kw, 2 * hp, :], p_e,
                                         start=(kt == 0), stop=(kt == NT - 1))
                        nc.tensor.matmul(o_o, vsb[kt][:kw, 2 * hp + 1, :], p_o,
                                         start=(kt == 0), stop=(kt == NT - 1))
                    nc.vector.tensor_copy(oraw_e[:, q0:q0 + w], o_e)
                    nc.vector.tensor_copy(oraw_o[:, q0:q0 + w], o_o)

                # denominators (row 64) -> reciprocal -> broadcast over the 64
                # head dims via DRAM-staged stride-0 DMA -> normalize.
                if dbg and hp == 0:
                    nc.sync.dma_start(out=d["dbg_oraw"][0], in_=oraw_e)
                    nc.sync.dma_start(out=d["dbg_oraw"][1], in_=oraw_o)
                # denominators (row 64) -> move to partitions 0-1 (the custom
                # DVE reciprocal only operates at partition base 0) -> one
                # batched reciprocal -> DRAM-staged stride-0 broadcast.
                den = obuf.tile([2, NP], F32, name="den")
                nc.sync.dma_start(out=den[0:1, :], in_=oraw_e[64:65, :])
                nc.sync.dma_start(out=den[1:2, :], in_=oraw_o[64:65, :])
                nc.vector.reciprocal_approx_fast(den, den)
                dtmp = dstage.tile([2, NP], F32, name="dtmp")
                nc.sync.dma_start(out=dtmp[:, :], in_=den)
                bc_e = obuf.tile([64, NP], F32, name="bc_e")
                bc_o = obuf.tile([64, NP], F32, name="bc_o")
                nc.gpsimd.dma_start(out=bc_e, in_=_bcast_row(dtmp[0:1, :], 64))
                nc.gpsimd.dma_start(out=bc_o, in_=_bcast_row(dtmp[1:2, :], 64))
                if dbg and hp == 0:
                    nc.sync.dma_start(out=d["dbg_norm"][0][:65], in_=oraw_e)
                    nc.sync.dma_start(out=d["dbg_norm"][1][:64], in_=bc_e)
                    nc.sync.dma_start(out=d["dbg_norm"][2][:64], in_=bc_o)
                nc.vector.tensor_mul(projrhs[hp][0:64, :], oraw_e[0:64, :], bc_e)
                onorm_o = obuf.tile([64, NP], F32R, name="onorm_o")
                nc.vector.tensor_mul(onorm_o, oraw_o[0:64, :], bc_o)
                if dbg and hp == 0:
                    nc.sync.dma_start(out=d["dbg_norm"][3][:64], in_=onorm_o.bitcast(F32))
                # odd head lives on partitions 64-127 of the proj rhs tile;
                # DMA is the only engine that can shift partition ranges.
                nc.sync.dma_start(out=projrhs[hp][64:128, :], in_=onorm_o)



        # ---------------- phase 3: output projection -----------------------
        with tc.tile_pool(name="ps3", bufs=4, space="PSUM") as ps3, \
             tc.tile_pool(name="ybuf", bufs=3) as ybuf:
            for m in range(6):
                for (q0, w) in QSLICES:
                    ps = ps3.tile([128, 512], F32, name="yps")[:, :w]
                    for kk in range(KT):
                        nc.tensor.matmul(
                            ps,
                            wproj[kk][:, m * 128:(m + 1) * 128],
                            projrhs[kk][:, q0:q0 + w],
                            start=(kk == 0), stop=(kk == KT - 1),
                        )
                    yt = ybuf.tile([128, 512], F32, name="yt")[:, :w]
                    nc.scalar.activation(yt, ps, AFT.Identity,
                                         bias=projbt[:, m:m + 1], scale=1.0)
                    nc.sync.dma_start(out=d["out"][m * 128:(m + 1) * 128, q0:q0 + w],
                                      in_=yt)


_NC = None


def build_nc(dbg=False):
    global _NC
    if _NC is None or dbg:
        nc = bacc.Bacc("TRN2", target_bir_lowering=False, debug=False)
        d = {
            "xt": nc.dram_tensor("xt", [C, NP], F32R, kind="ExternalInput").ap(),
            "wqkvt": nc.dram_tensor("wqkvt", [C, 3 * C], F32R, kind="ExternalInput").ap(),
            "wprojt": nc.dram_tensor("wprojt", [C, C], F32R, kind="ExternalInput").ap(),
            "s2t": nc.dram_tensor("s2t", [128, 128], F32R, kind="ExternalInput").ap(),
            "cost2": nc.dram_tensor("cost2", [128, NP], F32, kind="ExternalInput").ap(),
            "sint2": nc.dram_tensor("sint2", [128, NP], F32, kind="ExternalInput").ap(),
            "qkvbt": nc.dram_tensor("qkvbt", [128, 12], F32, kind="ExternalInput").ap(),
            "projbt": nc.dram_tensor("projbt", [128, 6], F32, kind="ExternalInput").ap(),
            "vb": nc.dram_tensor("vb", [1, C], F32R, kind="ExternalInput").ap(),
            "ones1": nc.dram_tensor("ones1", [1, 128], F32R, kind="ExternalInput").ap(),
            "vones": nc.dram_tensor("vones", [128, 12, 1], BF16, kind="ExternalInput").ap(),
            "vzrow": nc.dram_tensor("vzrow", [1, 12, 65], BF16, kind="ExternalInput").ap(),
            "out": nc.dram_tensor("out", [C, NP], F32, kind="ExternalOutput").ap(),
        }
        if dbg:
            d["dbg_qk"] = nc.dram_tensor("dbg_qk", [12, 128, NP], F32, kind="ExternalOutput").ap()
            d["dbg_v"] = nc.dram_tensor("dbg_v", [NT, 128, 12, 65], F32, kind="ExternalOutput").ap()
            d["dbg_oraw"] = nc.dram_tensor("dbg_oraw", [2, 65, NP], F32, kind="ExternalOutput").ap()
            d["dbg_prhs"] = nc.dram_tensor("dbg_prhs", [6, 128, NP], F32, kind="ExternalOutput").ap()
            d["dbg_norm"] = nc.dram_tensor("dbg_norm", [4, 65, NP], F32, kind="ExternalOutput").ap()
        with tile.TileContext(nc) as tc:
            _emit(nc, tc, d, dbg=dbg)
        nc.compile()
        if dbg:
            return nc
        _NC = nc
    return _NC


try:
    import ml_dtypes
    _bf16 = ml_dtypes.bfloat16
except ImportError:  # pragma: no cover
    _bf16 = np.float16


def _round_fp32r(a):
    """Round fp32 array to the fp32r grid (11-bit mantissa, RNE)."""
    u = np.ascontiguousarray(a, dtype=np.float32).view(np.uint32).copy()
    u += 0x7FF + ((u >> 12) & 1)
    u &= 0xFFFFF000
    return u.view(np.float32)


def make_in_maps(inputs):
    x = np.ascontiguousarray(np.asarray(inputs["x"], dtype=np.float32))
    sin = np.asarray(inputs["sin"], dtype=np.float32)
    cos = np.asarray(inputs["cos"], dtype=np.float32)
    qkv_w = np.asarray(inputs["qkv_w"], dtype=np.float32)
    qkv_b = np.asarray(inputs["qkv_b"], dtype=np.float32)
    proj_w = np.asarray(inputs["proj_w"], dtype=np.float32)
    proj_b = np.asarray(inputs["proj_b"], dtype=np.float32)

    # rotate_half as a signed permutation: rot = S64 @ q (per 64-dim head).
    s64 = np.zeros((64, 64), dtype=np.float32)
    for dd in range(32):
        s64[dd, dd + 32] = -1.0
        s64[dd + 32, dd] = 1.0
    s2t = np.zeros((128, 128), dtype=np.float32)
    s2t[:64, :64] = s64.T
    s2t[64:, 64:] = s64.T

    # [128, NP] sin/cos in [dim, token] layout, doubled for the head pair in
    # each 128-channel tile; col 0 (cls) and col 1025 (pad) get cos=1, sin=0.
    cost2 = np.ones((128, NP), dtype=np.float32)
    sint2 = np.zeros((128, NP), dtype=np.float32)
    cost2[:64, 1:N] = cos.T
    cost2[64:, 1:N] = cos.T
    sint2[:64, 1:N] = sin.T
    sint2[64:, 1:N] = sin.T

    shared = {
        "wqkvt": _round_fp32r(qkv_w.T),
        "wprojt": _round_fp32r(proj_w.T),
        "s2t": s2t,
        "cost2": cost2,
        "sint2": sint2,
        "qkvbt": np.ascontiguousarray(qkv_b[:2 * C].reshape(12, 128).T),
        "projbt": np.ascontiguousarray(proj_b.reshape(6, 128).T),
        "vb": _round_fp32r(qkv_b[2 * C:].reshape(1, C)),
        "ones1": np.ones((1, 128), dtype=np.float32),
        "vones": np.ones((128, 12, 1), dtype=_bf16),
        "vzrow": np.zeros((1, 12, 65), dtype=_bf16),
    }
    xp = np.zeros((C, NP), dtype=np.float32)
    maps = []
    for b in range(B):
        xp[:, :N] = x[b].T
        maps.append(dict(shared, xt=_round_fp32r(xp)))
    return maps


def kernel(**inputs) -> np.ndarray:
    nc = build_nc()
    in_maps = make_in_maps(inputs)
    res = run_bass_kernel_spmd(nc, in_maps, core_ids=list(range(B)))
    return np.stack([res.results[b]["out"][:, :N].T for b in range(B)]).astype(np.float32)


# revision 10
# speedup vs baseline: 1.2616x; 1.0764x over previous
"""Trainium2 Bass kernel for nn_Attention (B=8, N=1025, C=768, H=12).

Strategy: pure data-parallel over batch — each of the 8 NeuronCores runs the
full attention block for one batch element.  All device-side tensors live in
"transposed" [channel, token] layout so no on-device transposes are needed:

  qkvT[ch, tok] = wqkvT.T @ xT           (q,k parts; ch on partitions)
  RoPE via a signed-permutation matmul (rotate_half = S @ q) + DVE elementwise
  v[tok, ch]    = xT.T @ wvT             (tok on partitions, per-head 65-col
                                          blocks with a ones column appended)
  ST[k, q]      = kT.T @ qT              (per head; head PAIRS row-packed on
                                          the 128x128 PE array, K=64 each)
  P = exp(ST * scale)                    (ScalarE, no max subtraction --
                                          logits have std ~2, max ~11, safe)
  O_aug[65, q]  = v_aug.T @ P            (row 64 = softmax denominator via the
                                          ones column; accumulated in PSUM)
  normalize with 1/denom broadcast across partitions via a small DRAM-staged
  stride-0 DMA, then yT = wprojT.T @ oT.

Precision: projections (qkv, proj) run as float32r (TF32-like 11-bit-mantissa
fp32, full PE rate for even free dims >= 256; tokens padded 1025->1026 and
every f32r matmul input produced "rounded", either by f32r-typed DMA loads
with host pre-rounding or f32r-typed compute writes).  The attention inner
loop (scores, exp, AV) runs in bf16 operands with fp32 PSUM accumulation.

Work is emitted v-first, then per head pair (qkv -> RoPE -> attention) so the
ScalarE exp stream — the steady-state bottleneck — starts as early as
possible and overlaps the remaining projection matmuls.
"""

import numpy as np

import concourse.bass as bass
import concourse.bacc as bacc
import concourse.tile as tile
from concourse import mybir
from concourse.bass_utils import run_bass_kernel_spmd

B, N, C, H, HD = 8, 1025, 768, 12, 64
NP = 1026                # padded token count (fp32r needs even free dims)
SCALE = HD ** -0.5
KT = C // 128            # 6 contraction tiles over channels
NT = (NP + 127) // 128   # 9 token tiles (last holds 2 tokens: 1 real + 1 pad)
TAILW = NP - 128 * (NT - 1)
QSLICES = [(0, 342), (342, 342), (684, 342)]
F32 = mybir.dt.float32
F32R = mybir.dt.float32r
BF16 = mybir.dt.bfloat16
AFT = mybir.ActivationFunctionType
ALU = mybir.AluOpType


def _r(ap):
    return ap.bitcast(F32R)


def _bcast_row(row_ap, parts):
    """AP reading a [1, W] DRAM row replicated across `parts` partitions."""
    return bass.AP(
        tensor=row_ap.tensor,
        offset=row_ap.offset,
        ap=[[0, parts]] + list(row_ap.ap[1:]),
    )


def _emit(nc, tc, d):
    from contextlib import ExitStack

    with ExitStack() as ctx:
        const = ctx.enter_context(tc.tile_pool(name="const", bufs=1))
        s2t = const.tile([128, 128], F32R, name="s2t_sb")
        nc.sync.dma_start(out=s2t, in_=d["s2t"])
        cost2 = const.tile([128, NP], F32, name="cost2_sb")
        nc.sync.dma_start(out=cost2, in_=d["cost2"])
        sint2 = const.tile([128, NP], F32, name="sint2_sb")
        nc.sync.dma_start(out=sint2, in_=d["sint2"])
        qkvbt = const.tile([128, 12], F32, name="qkvbt_sb")
        nc.sync.dma_start(out=qkvbt, in_=d["qkvbt"])
        projbt = const.tile([128, 6], F32, name="projbt_sb")
        nc.sync.dma_start(out=projbt, in_=d["projbt"])
        vb = const.tile([1, C], F32R, name="vb_sb")
        nc.sync.dma_start(out=vb, in_=d["vb"])
        ones1 = const.tile([1, 128], F32R, name="ones1")
        nc.sync.dma_start(out=ones1, in_=d["ones1"])

        qk_pool = ctx.enter_context(tc.tile_pool(name="qkp", bufs=1))
        qkb = [qk_pool.tile([128, NP], BF16, name=f"qkb{m}") for m in range(12)]
        v_pool = ctx.enter_context(tc.tile_pool(name="vp", bufs=1))
        vsb = [v_pool.tile([128, 12, 65], BF16, name=f"v{t}") for t in range(NT)]

        att = ctx.enter_context(tc.tile_pool(name="att", bufs=1))
        projrhs = [att.tile([128, NP], F32R, name=f"prhs{hp}") for hp in range(6)]

        psA = ctx.enter_context(tc.tile_pool(name="psA", bufs=2, space="PSUM"))
        inner = ctx.enter_context(ExitStack())
        xw = inner.enter_context(tc.tile_pool(name="xw", bufs=1))
        wqkp = inner.enter_context(tc.tile_pool(name="wqkp", bufs=2))
        qkr_pool = inner.enter_context(tc.tile_pool(name="qkr", bufs=2))
        tmp1 = inner.enter_context(tc.tile_pool(name="tmp1", bufs=4))
        ppool = inner.enter_context(tc.tile_pool(name="ppool", bufs=3))
        obuf = inner.enter_context(tc.tile_pool(name="obuf", bufs=1))
        dstage = inner.enter_context(tc.tile_pool(name="dstage", bufs=2, space="DRAM"))
        # PSUM budget is exactly 8 banks:
        #   mm(2) + scores e/o (2+2) + O accumulators e/o (1+1) = 8
        psS = inner.enter_context(tc.tile_pool(name="psS", bufs=2, space="PSUM"))
        psO = inner.enter_context(tc.tile_pool(name="psO", bufs=1, space="PSUM"))

        # input DMAs: x and the V columns of wqkv first so the v matmuls (and
        # with them the first attention pairs) start as early as possible.
        # qk weight columns stream in per head pair; wproj loads at the end.
        xt = [xw.tile([128, NP], F32R, name=f"xt{kk}") for kk in range(KT)]
        wqv = [xw.tile([128, C], F32R, name=f"wqv{kk}") for kk in range(KT)]
        for kk in range(KT):
            nc.sync.dma_start(out=xt[kk], in_=d["xt"][kk * 128:(kk + 1) * 128, :])
            nc.sync.dma_start(out=wqv[kk], in_=d["wqkvt"][kk * 128:(kk + 1) * 128, 2 * C:])

        # ---- v in [tok, ch] layout, per-head 65-col blocks + ones column ---
        for t in range(NT):
            tw = 128 if t < NT - 1 else TAILW
            for vj, (v0, wv) in enumerate([(0, 512), (512, 256)]):
                ps = psA.tile([128, 512], F32, name="mmps")[:tw, :wv]
                nc.tensor.matmul(ps, ones1[:1, :tw], vb[:1, v0:v0 + wv],
                                 start=True, stop=False)
                for kk in range(KT):
                    nc.tensor.matmul(
                        ps,
                        xt[kk][:, t * 128:t * 128 + tw],
                        wqv[kk][:, v0:v0 + wv],
                        start=False, stop=(kk == KT - 1),
                    )
                nc.vector.tensor_copy(
                    vsb[t][:tw, vj * 8:vj * 8 + wv // 64, 0:64],
                    ps.rearrange("p (h dd) -> p h dd", dd=64),
                )
            # ones column for real tokens; the pad-token row of the tail tile
            # is fully zeroed (keeps denominators exact even w/ nonzero bias).
            if t < NT - 1:
                nc.sync.dma_start(out=vsb[t][:tw, :, 64:65], in_=d["vones"][:tw])
            else:
                nc.sync.dma_start(out=vsb[t][0:tw - 1, :, 64:65],
                                  in_=d["vones"][:tw - 1])
                nc.sync.dma_start(out=vsb[t][tw - 1:tw, :, :], in_=d["vzrow"])

        # ---- per head pair: qkv -> rope -> attention -----------------------
        for hp in range(6):
            for m in (hp, 6 + hp):
                wm = [wqkp.tile([128, 128], F32R, name=f"wqk{kk}") for kk in range(KT)]
                for kk in range(KT):
                    nc.sync.dma_start(
                        out=wm[kk],
                        in_=d["wqkvt"][kk * 128:(kk + 1) * 128, m * 128:(m + 1) * 128])
                qkr = qkr_pool.tile([128, NP], F32, name=f"qkr{'qk'[m >= 6]}")
                for (q0, w) in QSLICES:
                    ps = psA.tile([128, 512], F32, name="mmps")[:, :w]
                    for kk in range(KT):
                        nc.tensor.matmul(
                            ps,
                            wm[kk],
                            xt[kk][:, q0:q0 + w],
                            start=(kk == 0), stop=(kk == KT - 1),
                        )
                    # eviction + bias on DVE (keeps ScalarE free for exp)
                    nc.vector.tensor_scalar_add(
                        out=_r(qkr[:, q0:q0 + w]), in0=ps,
                        scalar1=qkvbt[:, m:m + 1],
                    )
                    # RoPE: rope = qk*cos + (S @ qk)*sin, written as bf16
                    rps = psA.tile([128, 512], F32, name="mmps")[:, :w]
                    nc.tensor.matmul(rps, s2t, _r(qkr[:, q0:q0 + w]),
                                     start=True, stop=True)
                    a_t = tmp1.tile([128, 342], F32, name="ropea")[:, :w]
                    nc.gpsimd.tensor_mul(a_t, qkr[:, q0:q0 + w], cost2[:, q0:q0 + w])
                    b_t = tmp1.tile([128, 342], F32, name="ropeb")[:, :w]
                    nc.vector.tensor_mul(b_t, rps, sint2[:, q0:q0 + w])
                    nc.vector.tensor_add(qkb[m][:, q0:q0 + w], a_t, b_t)

            oraw_e = obuf.tile([65, NP], F32, name="oraw_e")
            oraw_o = obuf.tile([65, NP], F32, name="oraw_o")
            for (q0, w) in QSLICES:
                o_e = psO.tile([65, 512], F32, name="o_e")[:, :w]
                o_o = psO.tile([65, 512], F32, name="o_o")[:, :w]
                for kt in range(NT):
                    kw = 128 if kt < NT - 1 else TAILW
                    k0 = kt * 128
                    s_e = psS.tile([128, 512], F32, name="s_e")[:kw, :w]
                    s_o = psS.tile([128, 512], F32, name="s_o")[:kw, :w]
                    # head pair row-packed: even head on PE rows 0-63, odd
                    # head on rows 64-127 (auto tile_position from slices).
                    nc.tensor.matmul(s_e, qkb[6 + hp][0:64, k0:k0 + kw],
                                     qkb[hp][0:64, q0:q0 + w],
                                     start=True, stop=True)
                    nc.tensor.matmul(s_o, qkb[6 + hp][64:128, k0:k0 + kw],
                                     qkb[hp][64:128, q0:q0 + w],
                                     start=True, stop=True)
                    p_e = ppool.tile([128, 342], BF16, name="p_e")[:kw, :w]
                    p_o = ppool.tile([128, 342], BF16, name="p_o")[:kw, :w]
                    nc.scalar.activation(p_e, s_e, AFT.Exp, bias=0.0, scale=SCALE)
                    nc.scalar.activation(p_o, s_o, AFT.Exp, bias=0.0, scale=SCALE)
                    nc.tensor.matmul(o_e, vsb[kt][:kw, 2 * hp, :], p_e,
                                     start=(kt == 0), stop=(kt == NT - 1))
                    nc.tensor.matmul(o_o, vsb[kt][:kw, 2 * hp + 1, :], p_o,
                                     start=(kt == 0), stop=(kt == NT - 1))
                nc.vector.tensor_copy(oraw_e[:, q0:q0 + w], o_e)
                nc.vector.tensor_copy(oraw_o[:, q0:q0 + w], o_o)

            # denominators (row 64) -> partitions 0-1 (the custom DVE
            # reciprocal only works at partition base 0) -> one batched
            # reciprocal -> DRAM-staged stride-0 partition broadcast.
            den = obuf.tile([2, NP], F32, name="den")
            nc.sync.dma_start(out=den[0:1, :], in_=oraw_e[64:65, :])
            nc.sync.dma_start(out=den[1:2, :], in_=oraw_o[64:65, :])
            nc.vector.reciprocal_approx_fast(den, den)
            dtmp = dstage.tile([2, NP], F32, name="dtmp")
            nc.sync.dma_start(out=dtmp[:, :], in_=den)
            bc_e = obuf.tile([64, NP], F32, name="bc_e")
            bc_o = obuf.tile([64, NP], F32, name="bc_o")
            nc.gpsimd.dma_start(out=bc_e, in_=_bcast_row(dtmp[0:1, :], 64))
            nc.gpsimd.dma_start(out=bc_o, in_=_bcast_row(dtmp[1:2, :], 64))
            nc.vector.tensor_mul(projrhs[hp][0:64, :], oraw_e[0:64, :], bc_e)
            onorm_o = obuf.tile([64, NP], F32R, name="onorm_o")
            nc.vector.tensor_mul(onorm_o, oraw_o[0:64, :], bc_o)
            # odd head lives on partitions 64-127 of the proj rhs tile;
            # DMA is the only engine that can shift partition ranges.
            nc.sync.dma_start(out=projrhs[hp][64:128, :], in_=onorm_o)

        # ---------------- output projection --------------------------------
        inner.close()
        with tc.tile_pool(name="wyb", bufs=1) as wyb, \
             tc.tile_pool(name="ybuf", bufs=3) as ybuf:
            wproj = [wyb.tile([128, C], F32R, name=f"wp{kk}") for kk in range(KT)]
            for kk in range(KT):
                nc.sync.dma_start(out=wproj[kk],
                                  in_=d["wprojt"][kk * 128:(kk + 1) * 128, :])
            for m in range(6):
                for (q0, w) in QSLICES:
                    ps = psA.tile([128, 512], F32, name="mmps")[:, :w]
                    for kk in range(KT):
                        nc.tensor.matmul(
                            ps,
                            wproj[kk][:, m * 128:(m + 1) * 128],
                            projrhs[kk][:, q0:q0 + w],
                            start=(kk == 0), stop=(kk == KT - 1),
                        )
                    yt = ybuf.tile([128, 512], F32, name="yt")[:, :w]
                    nc.scalar.activation(yt, ps, AFT.Identity,
                                         bias=projbt[:, m:m + 1], scale=1.0)
                    nc.sync.dma_start(out=d["out"][m * 128:(m + 1) * 128, q0:q0 + w],
                                      in_=yt)


_NC = None


def build_nc():
    global _NC
    if _NC is None:
        nc = bacc.Bacc("TRN2", target_bir_lowering=False, debug=False)
        d = {
            "xt": nc.dram_tensor("xt", [C, NP], F32R, kind="ExternalInput").ap(),
            "wqkvt": nc.dram_tensor("wqkvt", [C, 3 * C], F32R, kind="ExternalInput").ap(),
            "wprojt": nc.dram_tensor("wprojt", [C, C], F32R, kind="ExternalInput").ap(),
            "s2t": nc.dram_tensor("s2t", [128, 128], F32R, kind="ExternalInput").ap(),
            "cost2": nc.dram_tensor("cost2", [128, NP], F32, kind="ExternalInput").ap(),
            "sint2": nc.dram_tensor("sint2", [128, NP], F32, kind="ExternalInput").ap(),
            "qkvbt": nc.dram_tensor("qkvbt", [128, 12], F32, kind="ExternalInput").ap(),
            "projbt": nc.dram_tensor("projbt", [128, 6], F32, kind="ExternalInput").ap(),
            "vb": nc.dram_tensor("vb", [1, C], F32R, kind="ExternalInput").ap(),
            "ones1": nc.dram_tensor("ones1", [1, 128], F32R, kind="ExternalInput").ap(),
            "vones": nc.dram_tensor("vones", [128, 12, 1], BF16, kind="ExternalInput").ap(),
            "vzrow": nc.dram_tensor("vzrow", [1, 12, 65], BF16, kind="ExternalInput").ap(),
            "out": nc.dram_tensor("out", [C, NP], F32, kind="ExternalOutput").ap(),
        }
        with tile.TileContext(nc) as tc:
            _emit(nc, tc, d)
        nc.compile()
        _NC = nc
    return _NC


try:
    import ml_dtypes
    _bf16 = ml_dtypes.bfloat16
except ImportError:  # pragma: no cover
    _bf16 = np.float16


def _round_fp32r(a):
    """Round fp32 array to the fp32r grid (11-bit mantissa, RNE)."""
    u = np.ascontiguousarray(a, dtype=np.float32).view(np.uint32).copy()
    u += 0x7FF + ((u >> 12) & 1)
    u &= 0xFFFFF000
    return u.view(np.float32)


def make_in_maps(inputs):
    x = np.ascontiguousarray(np.asarray(inputs["x"], dtype=np.float32))
    sin = np.asarray(inputs["sin"], dtype=np.float32)
    cos = np.asarray(inputs["cos"], dtype=np.float32)
    qkv_w = np.asarray(inputs["qkv_w"], dtype=np.float32)
    qkv_b = np.asarray(inputs["qkv_b"], dtype=np.float32)
    proj_w = np.asarray(inputs["proj_w"], dtype=np.float32)
    proj_b = np.asarray(inputs["proj_b"], dtype=np.float32)

    # rotate_half as a signed permutation: rot = S64 @ q (per 64-dim head).
    s64 = np.zeros((64, 64), dtype=np.float32)
    for dd in range(32):
        s64[dd, dd + 32] = -1.0
        s64[dd + 32, dd] = 1.0
    s2t = np.zeros((128, 128), dtype=np.float32)
    s2t[:64, :64] = s64.T
    s2t[64:, 64:] = s64.T

    # [128, NP] sin/cos in [dim, token] layout, doubled for the head pair in
    # each 128-channel tile; col 0 (cls) and col 1025 (pad) get cos=1, sin=0.
    cost2 = np.ones((128, NP), dtype=np.float32)
    sint2 = np.zeros((128, NP), dtype=np.float32)
    cost2[:64, 1:N] = cos.T
    cost2[64:, 1:N] = cos.T
    sint2[:64, 1:N] = sin.T
    sint2[64:, 1:N] = sin.T

    shared = {
        "wqkvt": _round_fp32r(qkv_w.T),
        "wprojt": _round_fp32r(proj_w.T),
        "s2t": s2t,
        "cost2": cost2,
        "sint2": sint2,
        "qkvbt": np.ascontiguousarray(qkv_b[:2 * C].reshape(12, 128).T),
        "projbt": np.ascontiguousarray(proj_b.reshape(6, 128).T),
        "vb": _round_fp32r(qkv_b[2 * C:].reshape(1, C)),
        "ones1": np.ones((1, 128), dtype=np.float32),
        "vones": np.ones((128, 12, 1), dtype=_bf16),
        "vzrow": np.zeros((1, 12, 65), dtype=_bf16),
    }
    xp = np.zeros((C, NP), dtype=np.float32)
    maps = []
    for b in range(B):
        xp[:, :N] = x[b].T
        maps.append(dict(shared, xt=_round_fp32r(xp)))
    return maps


def kernel(**inputs) -> np.ndarray:
    nc = build_nc()
    in_maps = make_in_maps(inputs)
    res = run_bass_kernel_spmd(nc, in_maps, core_ids=list(range(B)))
    return np.stack([res.results[b]["out"][:, :N].T for b in range(B)]).astype(np.float32)


# revision 11
# speedup vs baseline: 1.3893x; 1.1012x over previous
"""Trainium2 Bass kernel for nn_Attention (B=8, N=1025, C=768, H=12).

Strategy: pure data-parallel over batch — each of the 8 NeuronCores runs the
full attention block for one batch element.  All device-side tensors live in
"transposed" [channel, token] layout so no on-device transposes are needed:

  qkvT[ch, tok] = wqkvT.T @ xT           (q,k parts; ch on partitions)
  RoPE via a signed-permutation matmul (rotate_half = S @ q) + DVE elementwise
  v[tok, ch]    = xT.T @ wvT             (tok on partitions, per-head 65-col
                                          blocks with a ones column appended)
  ST[k, q]      = kT.T @ qT              (per head; head PAIRS row-packed on
                                          the 128x128 PE array, K=64 each)
  P = exp(ST * scale)                    (ScalarE, no max subtraction --
                                          logits have std ~2, max ~11, safe)
  O_aug[65, q]  = v_aug.T @ P            (row 64 = softmax denominator via the
                                          ones column; accumulated in PSUM)
  normalize with 1/denom broadcast across partitions via a small DRAM-staged
  stride-0 DMA, then yT = wprojT.T @ oT.

Precision: projections (qkv, proj) run as float32r (TF32-like 11-bit-mantissa
fp32, full PE rate for even free dims >= 256; tokens padded 1025->1026 and
every f32r matmul input produced "rounded", either by f32r-typed DMA loads
with host pre-rounding or f32r-typed compute writes).  The attention inner
loop (scores, exp, AV) runs in bf16 operands with fp32 PSUM accumulation.

Work is emitted v-first, then per head pair (qkv -> RoPE -> attention) so the
ScalarE exp stream — the steady-state bottleneck — starts as early as
possible and overlaps the remaining projection matmuls.
"""

import numpy as np

import concourse.bass as bass
import concourse.bacc as bacc
import concourse.tile as tile
from concourse import mybir
from concourse.bass_utils import run_bass_kernel_spmd

B, N, C, H, HD = 8, 1025, 768, 12, 64
NP = 1026                # padded token count (fp32r needs even free dims)
SCALE = HD ** -0.5
KT = C // 128            # 6 contraction tiles over channels
NT = (NP + 127) // 128   # 9 token tiles (last holds 2 tokens: 1 real + 1 pad)
TAILW = NP - 128 * (NT - 1)
QSLICES = [(0, 342), (342, 342), (684, 342)]
F32 = mybir.dt.float32
F32R = mybir.dt.float32r
BF16 = mybir.dt.bfloat16
AFT = mybir.ActivationFunctionType
ALU = mybir.AluOpType


def _r(ap):
    return ap.bitcast(F32R)


def _bcast_row(row_ap, parts):
    """AP reading a [1, W] DRAM row replicated across `parts` partitions."""
    return bass.AP(
        tensor=row_ap.tensor,
        offset=row_ap.offset,
        ap=[[0, parts]] + list(row_ap.ap[1:]),
    )


def _emit(nc, tc, d):
    from contextlib import ExitStack

    with ExitStack() as ctx:
        const = ctx.enter_context(tc.tile_pool(name="const", bufs=1))
        s2t = const.tile([128, 128], BF16, name="s2t_sb")
        nc.sync.dma_start(out=s2t, in_=d["s2t"])
        cost2 = const.tile([128, NP], BF16, name="cost2_sb")
        nc.sync.dma_start(out=cost2, in_=d["cost2"])
        sint2 = const.tile([128, NP], BF16, name="sint2_sb")
        nc.sync.dma_start(out=sint2, in_=d["sint2"])
        qkvbt = const.tile([128, 12], F32, name="qkvbt_sb")
        nc.sync.dma_start(out=qkvbt, in_=d["qkvbt"])
        projbt = const.tile([128, 6], F32, name="projbt_sb")
        nc.sync.dma_start(out=projbt, in_=d["projbt"])
        vb = const.tile([1, C], BF16, name="vb_sb")
        nc.sync.dma_start(out=vb, in_=d["vb"])
        ones1 = const.tile([1, 128], BF16, name="ones1")
        nc.sync.dma_start(out=ones1, in_=d["ones1"])

        qk_pool = ctx.enter_context(tc.tile_pool(name="qkp", bufs=1))
        qkb = [qk_pool.tile([128, NP], BF16, name=f"qkb{m}") for m in range(12)]
        v_pool = ctx.enter_context(tc.tile_pool(name="vp", bufs=1))
        vsb = [v_pool.tile([128, 12, 65], BF16, name=f"v{t}") for t in range(NT)]

        att = ctx.enter_context(tc.tile_pool(name="att", bufs=1))
        projrhs = [att.tile([128, NP], BF16, name=f"prhs{hp}") for hp in range(6)]

        psA = ctx.enter_context(tc.tile_pool(name="psA", bufs=2, space="PSUM"))
        inner = ctx.enter_context(ExitStack())
        xw = inner.enter_context(tc.tile_pool(name="xw", bufs=1))
        wqkp = inner.enter_context(tc.tile_pool(name="wqkp", bufs=2))
        qkr_pool = inner.enter_context(tc.tile_pool(name="qkr", bufs=2))
        tmp1 = inner.enter_context(tc.tile_pool(name="tmp1", bufs=4))
        ppool = inner.enter_context(tc.tile_pool(name="ppool", bufs=3))
        obuf = inner.enter_context(tc.tile_pool(name="obuf", bufs=1))
        dstage = inner.enter_context(tc.tile_pool(name="dstage", bufs=2, space="DRAM"))
        # PSUM budget is exactly 8 banks:
        #   mm(2) + scores e/o (2+2) + O accumulators e/o (1+1) = 8
        psS = inner.enter_context(tc.tile_pool(name="psS", bufs=2, space="PSUM"))
        psO = inner.enter_context(tc.tile_pool(name="psO", bufs=1, space="PSUM"))

        # input DMAs: x and the V columns of wqkv first so the v matmuls (and
        # with them the first attention pairs) start as early as possible.
        # qk weight columns stream in per head pair; wproj loads at the end.
        xt = [xw.tile([128, NP], BF16, name=f"xt{kk}") for kk in range(KT)]
        wqv = [xw.tile([128, C], BF16, name=f"wqv{kk}") for kk in range(KT)]
        for kk in range(KT):
            nc.sync.dma_start(out=xt[kk], in_=d["xt"][kk * 128:(kk + 1) * 128, :])
            nc.sync.dma_start(out=wqv[kk], in_=d["wqkvt"][kk * 128:(kk + 1) * 128, 2 * C:])

        # ---- v in [tok, ch] layout, per-head 65-col blocks + ones column ---
        for t in range(NT):
            tw = 128 if t < NT - 1 else TAILW
            for vj, (v0, wv) in enumerate([(0, 512), (512, 256)]):
                ps = psA.tile([128, 512], F32, name="mmps")[:tw, :wv]
                nc.tensor.matmul(ps, ones1[:1, :tw], vb[:1, v0:v0 + wv],
                                 start=True, stop=False)
                for kk in range(KT):
                    nc.tensor.matmul(
                        ps,
                        xt[kk][:, t * 128:t * 128 + tw],
                        wqv[kk][:, v0:v0 + wv],
                        start=False, stop=(kk == KT - 1),
                    )
                nc.vector.tensor_copy(
                    vsb[t][:tw, vj * 8:vj * 8 + wv // 64, 0:64],
                    ps.rearrange("p (h dd) -> p h dd", dd=64),
                )
            # ones column for real tokens; the pad-token row of the tail tile
            # is fully zeroed (keeps denominators exact even w/ nonzero bias).
            if t < NT - 1:
                nc.sync.dma_start(out=vsb[t][:tw, :, 64:65], in_=d["vones"][:tw])
            else:
                nc.sync.dma_start(out=vsb[t][0:tw - 1, :, 64:65],
                                  in_=d["vones"][:tw - 1])
                nc.sync.dma_start(out=vsb[t][tw - 1:tw, :, :], in_=d["vzrow"])

        # ---- per head pair: qkv -> rope -> attention -----------------------
        for hp in range(6):
            for m in (hp, 6 + hp):
                wm = [wqkp.tile([128, 128], BF16, name=f"wqk{kk}") for kk in range(KT)]
                for kk in range(KT):
                    nc.sync.dma_start(
                        out=wm[kk],
                        in_=d["wqkvt"][kk * 128:(kk + 1) * 128, m * 128:(m + 1) * 128])
                qkr = qkr_pool.tile([128, NP], BF16, name=f"qkr{'qk'[m >= 6]}")
                for (q0, w) in QSLICES:
                    ps = psA.tile([128, 512], F32, name="mmps")[:, :w]
                    for kk in range(KT):
                        nc.tensor.matmul(
                            ps,
                            wm[kk],
                            xt[kk][:, q0:q0 + w],
                            start=(kk == 0), stop=(kk == KT - 1),
                        )
                    # eviction + bias on DVE (keeps ScalarE free for exp)
                    nc.vector.tensor_scalar_add(
                        out=qkr[:, q0:q0 + w], in0=ps,
                        scalar1=qkvbt[:, m:m + 1],
                    )
                    # RoPE: rope = qk*cos + (S @ qk)*sin, written as bf16
                    rps = psA.tile([128, 512], F32, name="mmps")[:, :w]
                    nc.tensor.matmul(rps, s2t, qkr[:, q0:q0 + w],
                                     start=True, stop=True)
                    a_t = tmp1.tile([128, 342], BF16, name="ropea")[:, :w]
                    nc.gpsimd.tensor_mul(a_t, qkr[:, q0:q0 + w], cost2[:, q0:q0 + w])
                    b_t = tmp1.tile([128, 342], BF16, name="ropeb")[:, :w]
                    nc.vector.tensor_mul(b_t, rps, sint2[:, q0:q0 + w])
                    nc.vector.tensor_add(qkb[m][:, q0:q0 + w], a_t, b_t)

            oraw_e = obuf.tile([65, NP], F32, name="oraw_e")
            oraw_o = obuf.tile([65, NP], F32, name="oraw_o")
            for (q0, w) in QSLICES:
                o_e = psO.tile([65, 512], F32, name="o_e")[:, :w]
                o_o = psO.tile([65, 512], F32, name="o_o")[:, :w]
                for kt in range(NT):
                    kw = 128 if kt < NT - 1 else TAILW
                    k0 = kt * 128
                    s_e = psS.tile([128, 512], F32, name="s_e")[:kw, :w]
                    s_o = psS.tile([128, 512], F32, name="s_o")[:kw, :w]
                    # head pair row-packed: even head on PE rows 0-63, odd
                    # head on rows 64-127 (auto tile_position from slices).
                    nc.tensor.matmul(s_e, qkb[6 + hp][0:64, k0:k0 + kw],
                                     qkb[hp][0:64, q0:q0 + w],
                                     start=True, stop=True)
                    nc.tensor.matmul(s_o, qkb[6 + hp][64:128, k0:k0 + kw],
                                     qkb[hp][64:128, q0:q0 + w],
                                     start=True, stop=True)
                    p_e = ppool.tile([128, 342], BF16, name="p_e")[:kw, :w]
                    p_o = ppool.tile([128, 342], BF16, name="p_o")[:kw, :w]
                    nc.scalar.activation(p_e, s_e, AFT.Exp, bias=0.0, scale=SCALE)
                    nc.scalar.activation(p_o, s_o, AFT.Exp, bias=0.0, scale=SCALE)
                    nc.tensor.matmul(o_e, vsb[kt][:kw, 2 * hp, :], p_e,
                                     start=(kt == 0), stop=(kt == NT - 1))
                    nc.tensor.matmul(o_o, vsb[kt][:kw, 2 * hp + 1, :], p_o,
                                     start=(kt == 0), stop=(kt == NT - 1))
                nc.vector.tensor_copy(oraw_e[:, q0:q0 + w], o_e)
                nc.vector.tensor_copy(oraw_o[:, q0:q0 + w], o_o)

            # denominators (row 64) -> partitions 0-1 (the custom DVE
            # reciprocal only works at partition base 0) -> one batched
            # reciprocal -> DRAM-staged stride-0 partition broadcast.
            den = obuf.tile([2, NP], F32, name="den")
            nc.sync.dma_start(out=den[0:1, :], in_=oraw_e[64:65, :])
            nc.sync.dma_start(out=den[1:2, :], in_=oraw_o[64:65, :])
            nc.vector.reciprocal_approx_fast(den, den)
            dtmp = dstage.tile([2, NP], F32, name="dtmp")
            nc.sync.dma_start(out=dtmp[:, :], in_=den)
            bc_e = obuf.tile([64, NP], F32, name="bc_e")
            bc_o = obuf.tile([64, NP], F32, name="bc_o")
            nc.gpsimd.dma_start(out=bc_e, in_=_bcast_row(dtmp[0:1, :], 64))
            nc.gpsimd.dma_start(out=bc_o, in_=_bcast_row(dtmp[1:2, :], 64))
            nc.vector.tensor_mul(projrhs[hp][0:64, :], oraw_e[0:64, :], bc_e)
            onorm_o = obuf.tile([64, NP], BF16, name="onorm_o")
            nc.vector.tensor_mul(onorm_o, oraw_o[0:64, :], bc_o)
            # odd head lives on partitions 64-127 of the proj rhs tile;
            # DMA is the only engine that can shift partition ranges.
            nc.sync.dma_start(out=projrhs[hp][64:128, :], in_=onorm_o)

        # ---------------- output projection --------------------------------
        inner.close()
        with tc.tile_pool(name="wyb", bufs=1) as wyb, \
             tc.tile_pool(name="ybuf", bufs=3) as ybuf:
            wproj = [wyb.tile([128, C], BF16, name=f"wp{kk}") for kk in range(KT)]
            for kk in range(KT):
                nc.sync.dma_start(out=wproj[kk],
                                  in_=d["wprojt"][kk * 128:(kk + 1) * 128, :])
            for m in range(6):
                for (q0, w) in QSLICES:
                    ps = psA.tile([128, 512], F32, name="mmps")[:, :w]
                    for kk in range(KT):
                        nc.tensor.matmul(
                            ps,
                            wproj[kk][:, m * 128:(m + 1) * 128],
                            projrhs[kk][:, q0:q0 + w],
                            start=(kk == 0), stop=(kk == KT - 1),
                        )
                    yt = ybuf.tile([128, 512], F32, name="yt")[:, :w]
                    nc.scalar.activation(yt, ps, AFT.Identity,
                                         bias=projbt[:, m:m + 1], scale=1.0)
                    nc.sync.dma_start(out=d["out"][m * 128:(m + 1) * 128, q0:q0 + w],
                                      in_=yt)


_NC = None


def build_nc():
    global _NC
    if _NC is None:
        nc = bacc.Bacc("TRN2", target_bir_lowering=False, debug=False)
        d = {
            "xt": nc.dram_tensor("xt", [C, NP], BF16, kind="ExternalInput").ap(),
            "wqkvt": nc.dram_tensor("wqkvt", [C, 3 * C], BF16, kind="ExternalInput").ap(),
            "wprojt": nc.dram_tensor("wprojt", [C, C], BF16, kind="ExternalInput").ap(),
            "s2t": nc.dram_tensor("s2t", [128, 128], BF16, kind="ExternalInput").ap(),
            "cost2": nc.dram_tensor("cost2", [128, NP], BF16, kind="ExternalInput").ap(),
            "sint2": nc.dram_tensor("sint2", [128, NP], BF16, kind="ExternalInput").ap(),
            "qkvbt": nc.dram_tensor("qkvbt", [128, 12], F32, kind="ExternalInput").ap(),
            "projbt": nc.dram_tensor("projbt", [128, 6], F32, kind="ExternalInput").ap(),
            "vb": nc.dram_tensor("vb", [1, C], BF16, kind="ExternalInput").ap(),
            "ones1": nc.dram_tensor("ones1", [1, 128], BF16, kind="ExternalInput").ap(),
            "vones": nc.dram_tensor("vones", [128, 12, 1], BF16, kind="ExternalInput").ap(),
            "vzrow": nc.dram_tensor("vzrow", [1, 12, 65], BF16, kind="ExternalInput").ap(),
            "out": nc.dram_tensor("out", [C, NP], F32, kind="ExternalOutput").ap(),
        }
        with tile.TileContext(nc) as tc:
            _emit(nc, tc, d)
        nc.compile()
        _NC = nc
    return _NC


try:
    import ml_dtypes
    _bf16 = ml_dtypes.bfloat16
except ImportError:  # pragma: no cover
    _bf16 = np.float16


def _round_fp32r(a):
    """Round fp32 array to the fp32r grid (11-bit mantissa, RNE)."""
    u = np.ascontiguousarray(a, dtype=np.float32).view(np.uint32).copy()
    u += 0x7FF + ((u >> 12) & 1)
    u &= 0xFFFFF000
    return u.view(np.float32)


def make_in_maps(inputs):
    x = np.ascontiguousarray(np.asarray(inputs["x"], dtype=np.float32))
    sin = np.asarray(inputs["sin"], dtype=np.float32)
    cos = np.asarray(inputs["cos"], dtype=np.float32)
    qkv_w = np.asarray(inputs["qkv_w"], dtype=np.float32)
    qkv_b = np.asarray(inputs["qkv_b"], dtype=np.float32)
    proj_w = np.asarray(inputs["proj_w"], dtype=np.float32)
    proj_b = np.asarray(inputs["proj_b"], dtype=np.float32)

    # rotate_half as a signed permutation: rot = S64 @ q (per 64-dim head).
    s64 = np.zeros((64, 64), dtype=np.float32)
    for dd in range(32):
        s64[dd, dd + 32] = -1.0
        s64[dd + 32, dd] = 1.0
    s2t = np.zeros((128, 128), dtype=np.float32)
    s2t[:64, :64] = s64.T
    s2t[64:, 64:] = s64.T

    # [128, NP] sin/cos in [dim, token] layout, doubled for the head pair in
    # each 128-channel tile; col 0 (cls) and col 1025 (pad) get cos=1, sin=0.
    cost2 = np.ones((128, NP), dtype=np.float32)
    sint2 = np.zeros((128, NP), dtype=np.float32)
    cost2[:64, 1:N] = cos.T
    cost2[64:, 1:N] = cos.T
    sint2[:64, 1:N] = sin.T
    sint2[64:, 1:N] = sin.T

    shared = {
        "wqkvt": np.ascontiguousarray(qkv_w.T).astype(_bf16),
        "wprojt": np.ascontiguousarray(proj_w.T).astype(_bf16),
        "s2t": s2t.astype(_bf16),
        "cost2": cost2.astype(_bf16),
        "sint2": sint2.astype(_bf16),
        "qkvbt": np.ascontiguousarray(qkv_b[:2 * C].reshape(12, 128).T),
        "projbt": np.ascontiguousarray(proj_b.reshape(6, 128).T),
        "vb": qkv_b[2 * C:].reshape(1, C).astype(_bf16),
        "ones1": np.ones((1, 128), dtype=_bf16),
        "vones": np.ones((128, 12, 1), dtype=_bf16),
        "vzrow": np.zeros((1, 12, 65), dtype=_bf16),
    }
    xp = np.zeros((C, NP), dtype=np.float32)
    maps = []
    for b in range(B):
        xp[:, :N] = x[b].T
        maps.append(dict(shared, xt=xp.astype(_bf16)))
    return maps


def kernel(**inputs) -> np.ndarray:
    nc = build_nc()
    in_maps = make_in_maps(inputs)
    res = run_bass_kernel_spmd(nc, in_maps, core_ids=list(range(B)))
    return np.stack([res.results[b]["out"][:, :N].T for b in range(B)]).astype(np.float32)


# revision 12
# speedup vs baseline: 1.7246x; 1.2413x over previous
"""Trainium2 Bass kernel for nn_Attention (B=8, N=1025, C=768, H=12).

Strategy: pure data-parallel over batch — each of the 8 NeuronCores runs the
full attention block for one batch element.  All device-side tensors live in
"transposed" [channel, token] layout so no on-device transposes are needed:

  qkvT[ch, tok] = wqkvT.T @ xT           (q,k parts; ch on partitions)
  RoPE via a signed-permutation matmul (rotate_half = S @ q) + DVE elementwise
  v[tok, ch]    = xT.T @ wvT             (tok on partitions, per-head 65-col
                                          blocks with a ones column appended)
  ST[k, q]      = kT.T @ qT              (per head; head PAIRS row-packed on
                                          the 128x128 PE array, K=64 each)
  P = exp(ST * scale)                    (ScalarE, no max subtraction --
                                          logits have std ~2, max ~11, safe)
  O_aug[65, q]  = v_aug.T @ P            (row 64 = softmax denominator via the
                                          ones column; accumulated in PSUM)
  normalize with 1/denom broadcast across partitions via a small DRAM-staged
  stride-0 DMA, then yT = wprojT.T @ oT.

Precision: projections (qkv, proj) run as float32r (TF32-like 11-bit-mantissa
fp32, full PE rate for even free dims >= 256; tokens padded 1025->1026 and
every f32r matmul input produced "rounded", either by f32r-typed DMA loads
with host pre-rounding or f32r-typed compute writes).  The attention inner
loop (scores, exp, AV) runs in bf16 operands with fp32 PSUM accumulation.

Work is emitted v-first, then per head pair (qkv -> RoPE -> attention) so the
ScalarE exp stream — the steady-state bottleneck — starts as early as
possible and overlaps the remaining projection matmuls.
"""

import numpy as np

import concourse.bass as bass
import concourse.bacc as bacc
import concourse.tile as tile
from concourse import mybir
from concourse.bass_utils import run_bass_kernel_spmd

B, N, C, H, HD = 8, 1025, 768, 12, 64
NP = 1026                # padded token count (fp32r needs even free dims)
SCALE = HD ** -0.5
KT = C // 128            # 6 contraction tiles over channels
NT = (NP + 127) // 128   # 9 token tiles (last holds 2 tokens: 1 real + 1 pad)
TAILW = NP - 128 * (NT - 1)
QSLICES = [(0, 342), (342, 342), (684, 342)]
F32 = mybir.dt.float32
F32R = mybir.dt.float32r
BF16 = mybir.dt.bfloat16
AFT = mybir.ActivationFunctionType
ALU = mybir.AluOpType


def _r(ap):
    return ap.bitcast(F32R)


def _bcast_row(row_ap, parts):
    """AP reading a [1, W] DRAM row replicated across `parts` partitions."""
    return bass.AP(
        tensor=row_ap.tensor,
        offset=row_ap.offset,
        ap=[[0, parts]] + list(row_ap.ap[1:]),
    )


def _emit(nc, tc, d):
    from contextlib import ExitStack

    with ExitStack() as ctx:
        const = ctx.enter_context(tc.tile_pool(name="const", bufs=1))
        s2t = const.tile([128, 128], BF16, name="s2t_sb")
        nc.sync.dma_start(out=s2t, in_=d["s2t"])
        cost2 = const.tile([128, NP], BF16, name="cost2_sb")
        nc.sync.dma_start(out=cost2, in_=d["cost2"])
        sint2 = const.tile([128, NP], BF16, name="sint2_sb")
        nc.sync.dma_start(out=sint2, in_=d["sint2"])
        qkvbt = const.tile([128, 12], F32, name="qkvbt_sb")
        nc.sync.dma_start(out=qkvbt, in_=d["qkvbt"])
        projbt = const.tile([128, 6], F32, name="projbt_sb")
        nc.sync.dma_start(out=projbt, in_=d["projbt"])
        vb = const.tile([1, C], BF16, name="vb_sb")
        nc.sync.dma_start(out=vb, in_=d["vb"])
        ones1 = const.tile([1, 128], BF16, name="ones1")
        nc.sync.dma_start(out=ones1, in_=d["ones1"])

        qk_pool = ctx.enter_context(tc.tile_pool(name="qkp", bufs=1))
        qkb = [qk_pool.tile([128, NP], BF16, name=f"qkb{m}") for m in range(12)]
        v_pool = ctx.enter_context(tc.tile_pool(name="vp", bufs=1))
        vsb = [v_pool.tile([128, 12, 65], BF16, name=f"v{t}") for t in range(NT)]

        att = ctx.enter_context(tc.tile_pool(name="att", bufs=1))
        projrhs = [att.tile([128, NP], BF16, name=f"prhs{hp}") for hp in range(6)]

        psA = ctx.enter_context(tc.tile_pool(name="psA", bufs=2, space="PSUM"))
        inner = ctx.enter_context(ExitStack())
        xw = inner.enter_context(tc.tile_pool(name="xw", bufs=1))
        wqkp = inner.enter_context(tc.tile_pool(name="wqkp", bufs=2))
        qkr_pool = inner.enter_context(tc.tile_pool(name="qkr", bufs=2))
        tmp1 = inner.enter_context(tc.tile_pool(name="tmp1", bufs=4))
        ppool = inner.enter_context(tc.tile_pool(name="ppool", bufs=3))
        obuf = inner.enter_context(tc.tile_pool(name="obuf", bufs=1))
        dstage = inner.enter_context(tc.tile_pool(name="dstage", bufs=2, space="DRAM"))
        # PSUM budget is exactly 8 banks:
        #   mm(2) + scores e/o (2+2) + O accumulators e/o (1+1) = 8
        psS = inner.enter_context(tc.tile_pool(name="psS", bufs=2, space="PSUM"))
        psO = inner.enter_context(tc.tile_pool(name="psO", bufs=1, space="PSUM"))

        # input DMAs: x and the V columns of wqkv first so the v matmuls (and
        # with them the first attention pairs) start as early as possible.
        # qk weight columns stream in per head pair; wproj loads at the end.
        xt = [xw.tile([128, NP], BF16, name=f"xt{kk}") for kk in range(KT)]
        wqv = [xw.tile([128, C], BF16, name=f"wqv{kk}") for kk in range(KT)]
        for kk in range(KT):
            nc.sync.dma_start(out=xt[kk], in_=d["xt"][kk * 128:(kk + 1) * 128, :])
            nc.sync.dma_start(out=wqv[kk], in_=d["wqkvt"][kk * 128:(kk + 1) * 128, 2 * C:])

        # ---- v in [tok, ch] layout, per-head 65-col blocks + ones column ---
        for t in range(NT):
            tw = 128 if t < NT - 1 else TAILW
            for vj, (v0, wv) in enumerate([(0, 512), (512, 256)]):
                ps = psA.tile([128, 512], F32, name="mmps")[:tw, :wv]
                nc.tensor.matmul(ps, ones1[:1, :tw], vb[:1, v0:v0 + wv],
                                 start=True, stop=False)
                for kk in range(KT):
                    nc.tensor.matmul(
                        ps,
                        xt[kk][:, t * 128:t * 128 + tw],
                        wqv[kk][:, v0:v0 + wv],
                        start=False, stop=(kk == KT - 1),
                    )
                nc.vector.tensor_copy(
                    vsb[t][:tw, vj * 8:vj * 8 + wv // 64, 0:64],
                    ps.rearrange("p (h dd) -> p h dd", dd=64),
                )
            # ones column for real tokens; the pad-token row of the tail tile
            # is fully zeroed (keeps denominators exact even w/ nonzero bias).
            if t < NT - 1:
                nc.sync.dma_start(out=vsb[t][:tw, :, 64:65], in_=d["vones"][:tw])
            else:
                nc.sync.dma_start(out=vsb[t][0:tw - 1, :, 64:65],
                                  in_=d["vones"][:tw - 1])
                nc.sync.dma_start(out=vsb[t][tw - 1:tw, :, :], in_=d["vzrow"])

        # ---- per head pair: qkv -> rope -> attention -----------------------
        for hp in range(6):
            for m in (hp, 6 + hp):
                wm = [wqkp.tile([128, 128], BF16, name=f"wqk{kk}") for kk in range(KT)]
                for kk in range(KT):
                    nc.sync.dma_start(
                        out=wm[kk],
                        in_=d["wqkvt"][kk * 128:(kk + 1) * 128, m * 128:(m + 1) * 128])
                qkr = qkr_pool.tile([128, NP], BF16, name=f"qkr{'qk'[m >= 6]}")
                for (q0, w) in QSLICES:
                    ps = psA.tile([128, 512], F32, name="mmps")[:, :w]
                    for kk in range(KT):
                        nc.tensor.matmul(
                            ps,
                            wm[kk],
                            xt[kk][:, q0:q0 + w],
                            start=(kk == 0), stop=(kk == KT - 1),
                        )
                    # eviction + bias on DVE (keeps ScalarE free for exp)
                    nc.vector.tensor_scalar_add(
                        out=qkr[:, q0:q0 + w], in0=ps,
                        scalar1=qkvbt[:, m:m + 1],
                    )
                    # RoPE: rope = qk*cos + (S @ qk)*sin, written as bf16
                    rps = psA.tile([128, 512], F32, name="mmps")[:, :w]
                    nc.tensor.matmul(rps, s2t, qkr[:, q0:q0 + w],
                                     start=True, stop=True)
                    a_t = tmp1.tile([128, 342], BF16, name="ropea")[:, :w]
                    nc.gpsimd.tensor_mul(a_t, qkr[:, q0:q0 + w], cost2[:, q0:q0 + w])
                    b_t = tmp1.tile([128, 342], BF16, name="ropeb")[:, :w]
                    nc.vector.tensor_mul(b_t, rps, sint2[:, q0:q0 + w])
                    nc.vector.tensor_add(qkb[m][:, q0:q0 + w], a_t, b_t)

            oraw_e = obuf.tile([65, NP], F32, name="oraw_e")
            oraw_o = obuf.tile([65, NP], F32, name="oraw_o")
            for (q0, w) in QSLICES:
                o_e = psO.tile([65, 512], F32, name="o_e")[:, :w]
                o_o = psO.tile([65, 512], F32, name="o_o")[:, :w]
                for kt in range(NT):
                    kw = 128 if kt < NT - 1 else TAILW
                    k0 = kt * 128
                    # both heads' score tiles live in ONE 2-bank psum tile so
                    # the row-packed pair issues back-to-back (concurrent on
                    # the PE: even head rows 0-63, odd head rows 64-127) and
                    # a single strided exp consumes both.
                    s2b = psS.tile([128, 1024], F32, name="s2b")
                    nc.tensor.matmul(s2b[:kw, 0:w], qkb[6 + hp][0:64, k0:k0 + kw],
                                     qkb[hp][0:64, q0:q0 + w],
                                     start=True, stop=True)
                    nc.tensor.matmul(s2b[:kw, 512:512 + w],
                                     qkb[6 + hp][64:128, k0:k0 + kw],
                                     qkb[hp][64:128, q0:q0 + w],
                                     start=True, stop=True)
                    p2 = ppool.tile([128, 2, 342], BF16, name="p2")[:kw, :, :w]
                    nc.scalar.activation(
                        p2,
                        s2b[:kw].rearrange("p (b c) -> p b c", b=2)[:, :, :w],
                        AFT.Exp, bias=0.0, scale=SCALE)
                    nc.tensor.matmul(o_e, vsb[kt][:kw, 2 * hp, :], p2[:, 0, :],
                                     start=(kt == 0), stop=(kt == NT - 1))
                    nc.tensor.matmul(o_o, vsb[kt][:kw, 2 * hp + 1, :], p2[:, 1, :],
                                     start=(kt == 0), stop=(kt == NT - 1))
                nc.vector.tensor_copy(oraw_e[:, q0:q0 + w], o_e)
                nc.vector.tensor_copy(oraw_o[:, q0:q0 + w], o_o)

            # denominators (row 64) -> partitions 0-1 (the custom DVE
            # reciprocal only works at partition base 0) -> one batched
            # reciprocal -> DRAM-staged stride-0 partition broadcast.
            den = obuf.tile([2, NP], F32, name="den")
            nc.sync.dma_start(out=den[0:1, :], in_=oraw_e[64:65, :])
            nc.sync.dma_start(out=den[1:2, :], in_=oraw_o[64:65, :])
            nc.vector.reciprocal_approx_fast(den, den)
            dtmp = dstage.tile([2, NP], F32, name="dtmp")
            nc.sync.dma_start(out=dtmp[:, :], in_=den)
            bc_e = obuf.tile([64, NP], F32, name="bc_e")
            bc_o = obuf.tile([64, NP], F32, name="bc_o")
            nc.gpsimd.dma_start(out=bc_e, in_=_bcast_row(dtmp[0:1, :], 64))
            nc.gpsimd.dma_start(out=bc_o, in_=_bcast_row(dtmp[1:2, :], 64))
            nc.vector.tensor_mul(projrhs[hp][0:64, :], oraw_e[0:64, :], bc_e)
            onorm_o = obuf.tile([64, NP], BF16, name="onorm_o")
            nc.vector.tensor_mul(onorm_o, oraw_o[0:64, :], bc_o)
            # odd head lives on partitions 64-127 of the proj rhs tile;
            # DMA is the only engine that can shift partition ranges.
            nc.sync.dma_start(out=projrhs[hp][64:128, :], in_=onorm_o)

        # ---------------- output projection --------------------------------
        inner.close()
        with tc.tile_pool(name="wyb", bufs=1) as wyb, \
             tc.tile_pool(name="ybuf", bufs=3) as ybuf:
            wproj = [wyb.tile([128, C], BF16, name=f"wp{kk}") for kk in range(KT)]
            for kk in range(KT):
                nc.sync.dma_start(out=wproj[kk],
                                  in_=d["wprojt"][kk * 128:(kk + 1) * 128, :])
            for m in range(6):
                for (q0, w) in QSLICES:
                    ps = psA.tile([128, 512], F32, name="mmps")[:, :w]
                    for kk in range(KT):
                        nc.tensor.matmul(
                            ps,
                            wproj[kk][:, m * 128:(m + 1) * 128],
                            projrhs[kk][:, q0:q0 + w],
                            start=(kk == 0), stop=(kk == KT - 1),
                        )
                    yt = ybuf.tile([128, 512], F32, name="yt")[:, :w]
                    nc.scalar.activation(yt, ps, AFT.Identity,
                                         bias=projbt[:, m:m + 1], scale=1.0)
                    nc.sync.dma_start(out=d["out"][m * 128:(m + 1) * 128, q0:q0 + w],
                                      in_=yt)


_NC = None


def build_nc():
    global _NC
    if _NC is None:
        nc = bacc.Bacc("TRN2", target_bir_lowering=False, debug=False)
        d = {
            "xt": nc.dram_tensor("xt", [C, NP], BF16, kind="ExternalInput").ap(),
            "wqkvt": nc.dram_tensor("wqkvt", [C, 3 * C], BF16, kind="ExternalInput").ap(),
            "wprojt": nc.dram_tensor("wprojt", [C, C], BF16, kind="ExternalInput").ap(),
            "s2t": nc.dram_tensor("s2t", [128, 128], BF16, kind="ExternalInput").ap(),
            "cost2": nc.dram_tensor("cost2", [128, NP], BF16, kind="ExternalInput").ap(),
            "sint2": nc.dram_tensor("sint2", [128, NP], BF16, kind="ExternalInput").ap(),
            "qkvbt": nc.dram_tensor("qkvbt", [128, 12], F32, kind="ExternalInput").ap(),
            "projbt": nc.dram_tensor("projbt", [128, 6], F32, kind="ExternalInput").ap(),
            "vb": nc.dram_tensor("vb", [1, C], BF16, kind="ExternalInput").ap(),
            "ones1": nc.dram_tensor("ones1", [1, 128], BF16, kind="ExternalInput").ap(),
            "vones": nc.dram_tensor("vones", [128, 12, 1], BF16, kind="ExternalInput").ap(),
            "vzrow": nc.dram_tensor("vzrow", [1, 12, 65], BF16, kind="ExternalInput").ap(),
            "out": nc.dram_tensor("out", [C, NP], F32, kind="ExternalOutput").ap(),
        }
        with tile.TileContext(nc) as tc:
            _emit(nc, tc, d)
        nc.compile()
        _NC = nc
    return _NC


try:
    import ml_dtypes
    _bf16 = ml_dtypes.bfloat16
except ImportError:  # pragma: no cover
    _bf16 = np.float16


def _round_fp32r(a):
    """Round fp32 array to the fp32r grid (11-bit mantissa, RNE)."""
    u = np.ascontiguousarray(a, dtype=np.float32).view(np.uint32).copy()
    u += 0x7FF + ((u >> 12) & 1)
    u &= 0xFFFFF000
    return u.view(np.float32)


def make_in_maps(inputs):
    x = np.ascontiguousarray(np.asarray(inputs["x"], dtype=np.float32))
    sin = np.asarray(inputs["sin"], dtype=np.float32)
    cos = np.asarray(inputs["cos"], dtype=np.float32)
    qkv_w = np.asarray(inputs["qkv_w"], dtype=np.float32)
    qkv_b = np.asarray(inputs["qkv_b"], dtype=np.float32)
    proj_w = np.asarray(inputs["proj_w"], dtype=np.float32)
    proj_b = np.asarray(inputs["proj_b"], dtype=np.float32)

    # rotate_half as a signed permutation: rot = S64 @ q (per 64-dim head).
    s64 = np.zeros((64, 64), dtype=np.float32)
    for dd in range(32):
        s64[dd, dd + 32] = -1.0
        s64[dd + 32, dd] = 1.0
    s2t = np.zeros((128, 128), dtype=np.float32)
    s2t[:64, :64] = s64.T
    s2t[64:, 64:] = s64.T

    # [128, NP] sin/cos in [dim, token] layout, doubled for the head pair in
    # each 128-channel tile; col 0 (cls) and col 1025 (pad) get cos=1, sin=0.
    cost2 = np.ones((128, NP), dtype=np.float32)
    sint2 = np.zeros((128, NP), dtype=np.float32)
    cost2[:64, 1:N] = cos.T
    cost2[64:, 1:N] = cos.T
    sint2[:64, 1:N] = sin.T
    sint2[64:, 1:N] = sin.T

    shared = {
        "wqkvt": np.ascontiguousarray(qkv_w.T).astype(_bf16),
        "wprojt": np.ascontiguousarray(proj_w.T).astype(_bf16),
        "s2t": s2t.astype(_bf16),
        "cost2": cost2.astype(_bf16),
        "sint2": sint2.astype(_bf16),
        "qkvbt": np.ascontiguousarray(qkv_b[:2 * C].reshape(12, 128).T),
        "projbt": np.ascontiguousarray(proj_b.reshape(6, 128).T),
        "vb": qkv_b[2 * C:].reshape(1, C).astype(_bf16),
        "ones1": np.ones((1, 128), dtype=_bf16),
        "vones": np.ones((128, 12, 1), dtype=_bf16),
        "vzrow": np.zeros((1, 12, 65), dtype=_bf16),
    }
    xp = np.zeros((C, NP), dtype=np.float32)
    maps = []
    for b in range(B):
        xp[:, :N] = x[b].T
        maps.append(dict(shared, xt=xp.astype(_bf16)))
    return maps


def kernel(**inputs) -> np.ndarray:
    nc = build_nc()
    in_maps = make_in_maps(inputs)
    res = run_bass_kernel_spmd(nc, in_maps, core_ids=list(range(B)))
    return np.stack([res.results[b]["out"][:, :N].T for b in range(B)]).astype(np.float32)


# revision 14
# speedup vs baseline: 1.9829x; 1.1497x over previous
"""Trainium2 Bass kernel for nn_Attention (B=8, N=1025, C=768, H=12).

Strategy: pure data-parallel over batch — each of the 8 NeuronCores runs the
full attention block for one batch element.  All device-side tensors live in
"transposed" [channel, token] layout so no on-device transposes are needed:

  qkvT[ch, tok] = wqkvT.T @ xT           (q,k parts; ch on partitions)
  RoPE via a signed-permutation matmul (rotate_half = S @ q) + DVE elementwise
  v[tok, ch]    = xT.T @ wvT             (tok on partitions, per-head 65-col
                                          blocks with a ones column appended)
  ST[k, q]      = kT.T @ qT              (per head; head PAIRS row-packed on
                                          the 128x128 PE array, K=64 each)
  P = exp(ST * scale)                    (ScalarE, no max subtraction --
                                          logits have std ~2, max ~11, safe)
  O_aug[65, q]  = v_aug.T @ P            (row 64 = softmax denominator via the
                                          ones column; accumulated in PSUM)
  normalize with 1/denom broadcast across partitions via a small DRAM-staged
  stride-0 DMA, then yT = wprojT.T @ oT.

Precision: projections (qkv, proj) run as float32r (TF32-like 11-bit-mantissa
fp32, full PE rate for even free dims >= 256; tokens padded 1025->1026 and
every f32r matmul input produced "rounded", either by f32r-typed DMA loads
with host pre-rounding or f32r-typed compute writes).  The attention inner
loop (scores, exp, AV) runs in bf16 operands with fp32 PSUM accumulation.

Work is emitted v-first, then per head pair (qkv -> RoPE -> attention) so the
ScalarE exp stream — the steady-state bottleneck — starts as early as
possible and overlaps the remaining projection matmuls.
"""

import numpy as np

import concourse.bass as bass
import concourse.bacc as bacc
import concourse.tile as tile
from concourse import mybir
from concourse.bass_utils import run_bass_kernel_spmd

B, N, C, H, HD = 8, 1025, 768, 12, 64
NP = 1026                # padded token count (fp32r needs even free dims)
SCALE = HD ** -0.5
KT = C // 128            # 6 contraction tiles over channels
NT = (NP + 127) // 128   # 9 token tiles (last holds 2 tokens: 1 real + 1 pad)
TAILW = NP - 128 * (NT - 1)
QSLICES = [(0, 342), (342, 342), (684, 342)]
F32 = mybir.dt.float32
F32R = mybir.dt.float32r
BF16 = mybir.dt.bfloat16
AFT = mybir.ActivationFunctionType
ALU = mybir.AluOpType


def _r(ap):
    return ap.bitcast(F32R)


def _bcast_row(row_ap, parts):
    """AP reading a [1, W] DRAM row replicated across `parts` partitions."""
    return bass.AP(
        tensor=row_ap.tensor,
        offset=row_ap.offset,
        ap=[[0, parts]] + list(row_ap.ap[1:]),
    )


def _emit(nc, tc, d):
    from contextlib import ExitStack

    with ExitStack() as ctx:
        const = ctx.enter_context(tc.tile_pool(name="const", bufs=1))
        s2t = const.tile([128, 128], F32R, name="s2t_sb")
        nc.sync.dma_start(out=s2t, in_=d["s2t"])
        cost2 = const.tile([128, NP], F32, name="cost2_sb")
        nc.sync.dma_start(out=cost2, in_=d["cost2"])
        sint2 = const.tile([128, NP], F32, name="sint2_sb")
        nc.sync.dma_start(out=sint2, in_=d["sint2"])
        qkvbt = const.tile([128, 12], F32, name="qkvbt_sb")
        nc.sync.dma_start(out=qkvbt, in_=d["qkvbt"])
        projbt = const.tile([128, 6], F32, name="projbt_sb")
        nc.sync.dma_start(out=projbt, in_=d["projbt"])
        vb = const.tile([1, C], BF16, name="vb_sb")
        nc.sync.dma_start(out=vb, in_=d["vb"])
        ones1 = const.tile([1, 128], BF16, name="ones1")
        nc.sync.dma_start(out=ones1, in_=d["ones1"])

        qk_pool = ctx.enter_context(tc.tile_pool(name="qkp", bufs=1))
        qkb = [qk_pool.tile([128, NP], BF16, name=f"qkb{m}") for m in range(12)]
        v_pool = ctx.enter_context(tc.tile_pool(name="vp", bufs=1))
        vsb = [v_pool.tile([128, 12, 65], BF16, name=f"v{t}") for t in range(NT)]

        att = ctx.enter_context(tc.tile_pool(name="att", bufs=1))
        projrhs = [att.tile([128, NP], F32R, name=f"prhs{hp}") for hp in range(6)]

        psA = ctx.enter_context(tc.tile_pool(name="psA", bufs=2, space="PSUM"))
        inner = ctx.enter_context(ExitStack())
        xw = inner.enter_context(tc.tile_pool(name="xw", bufs=1))
        wqkp = inner.enter_context(tc.tile_pool(name="wqkp", bufs=2))
        qkr_pool = inner.enter_context(tc.tile_pool(name="qkr", bufs=2))
        tmp1 = inner.enter_context(tc.tile_pool(name="tmp1", bufs=4))
        ppool = inner.enter_context(tc.tile_pool(name="ppool", bufs=3))
        obuf = inner.enter_context(tc.tile_pool(name="obuf", bufs=2))
        dstage = inner.enter_context(tc.tile_pool(name="dstage", bufs=2, space="DRAM"))
        # PSUM budget is exactly 8 banks:
        #   mm(2) + scores e/o (2+2) + O accumulators e/o (1+1) = 8
        psS = inner.enter_context(tc.tile_pool(name="psS", bufs=2, space="PSUM"))
        psO = inner.enter_context(tc.tile_pool(name="psO", bufs=1, space="PSUM"))

        # input DMAs: x and the V columns of wqkv first so the v matmuls (and
        # with them the first attention pairs) start as early as possible.
        # qk weight columns stream in per head pair; wproj loads at the end.
        xt = [xw.tile([128, NP], BF16, name=f"xt{kk}") for kk in range(KT)]
        wqv = [xw.tile([128, C], BF16, name=f"wqv{kk}") for kk in range(KT)]
        for kk in range(KT):
            nc.sync.dma_start(out=xt[kk], in_=d["xt"][kk * 128:(kk + 1) * 128, :])
            nc.sync.dma_start(out=wqv[kk], in_=d["wqkvt"][kk * 128:(kk + 1) * 128, 2 * C:])

        # ---- v in [tok, ch] layout, per-head 65-col blocks + ones column ---
        for t in range(NT):
            tw = 128 if t < NT - 1 else TAILW
            for vj, (v0, wv) in enumerate([(0, 512), (512, 256)]):
                ps = psA.tile([128, 512], F32, name="mmps")[:tw, :wv]
                nc.tensor.matmul(ps, ones1[:1, :tw], vb[:1, v0:v0 + wv],
                                 start=True, stop=False)
                for kk in range(KT):
                    nc.tensor.matmul(
                        ps,
                        xt[kk][:, t * 128:t * 128 + tw],
                        wqv[kk][:, v0:v0 + wv],
                        start=False, stop=(kk == KT - 1),
                    )
                nc.vector.tensor_copy(
                    vsb[t][:tw, vj * 8:vj * 8 + wv // 64, 0:64],
                    ps.rearrange("p (h dd) -> p h dd", dd=64),
                )
            # ones column for real tokens; the pad-token row of the tail tile
            # is fully zeroed (keeps denominators exact even w/ nonzero bias).
            if t < NT - 1:
                nc.sync.dma_start(out=vsb[t][:tw, :, 64:65], in_=d["vones"][:tw])
            else:
                nc.sync.dma_start(out=vsb[t][0:tw - 1, :, 64:65],
                                  in_=d["vones"][:tw - 1])
                nc.sync.dma_start(out=vsb[t][tw - 1:tw, :, :], in_=d["vzrow"])

        # ---- per head pair: qkv -> rope -> attention -----------------------
        for hp in range(6):
            for m in (hp, 6 + hp):
                wm = [wqkp.tile([128, 128], BF16, name=f"wqk{kk}") for kk in range(KT)]
                for kk in range(KT):
                    nc.sync.dma_start(
                        out=wm[kk],
                        in_=d["wqkvt"][kk * 128:(kk + 1) * 128, m * 128:(m + 1) * 128])
                qkr = qkr_pool.tile([128, NP], F32, name=f"qkr{'qk'[m >= 6]}")
                for (q0, w) in QSLICES:
                    ps = psA.tile([128, 512], F32, name="mmps")[:, :w]
                    for kk in range(KT):
                        nc.tensor.matmul(
                            ps,
                            wm[kk],
                            xt[kk][:, q0:q0 + w],
                            start=(kk == 0), stop=(kk == KT - 1),
                        )
                    # eviction + bias on DVE (keeps ScalarE free for exp)
                    nc.vector.tensor_scalar_add(
                        out=_r(qkr[:, q0:q0 + w]), in0=ps,
                        scalar1=qkvbt[:, m:m + 1],
                    )
                    # RoPE: rope = qk*cos + (S @ qk)*sin, written as bf16
                    rps = psA.tile([128, 512], F32, name="mmps")[:, :w]
                    nc.tensor.matmul(rps, s2t, _r(qkr[:, q0:q0 + w]),
                                     start=True, stop=True)
                    a_t = tmp1.tile([128, 342], F32, name="ropea")[:, :w]
                    nc.gpsimd.tensor_mul(a_t, qkr[:, q0:q0 + w], cost2[:, q0:q0 + w])
                    b_t = tmp1.tile([128, 342], F32, name="ropeb")[:, :w]
                    nc.vector.tensor_mul(b_t, rps, sint2[:, q0:q0 + w])
                    nc.vector.tensor_add(qkb[m][:, q0:q0 + w], a_t, b_t)

            oraw_e = obuf.tile([65, NP], F32, name="oraw_e")
            oraw_o = obuf.tile([65, NP], F32, name="oraw_o")
            for (q0, w) in QSLICES:
                o_e = psO.tile([65, 512], F32, name="o_e")[:, :w]
                o_o = psO.tile([65, 512], F32, name="o_o")[:, :w]
                for kt in range(NT):
                    kw = 128 if kt < NT - 1 else TAILW
                    k0 = kt * 128
                    # both heads' score tiles live in ONE 2-bank psum tile so
                    # the row-packed pair issues back-to-back (concurrent on
                    # the PE: even head rows 0-63, odd head rows 64-127) and
                    # a single strided exp consumes both.
                    s2b = psS.tile([128, 1024], F32, name="s2b")
                    nc.tensor.matmul(s2b[:kw, 0:w], qkb[6 + hp][0:64, k0:k0 + kw],
                                     qkb[hp][0:64, q0:q0 + w],
                                     start=True, stop=True)
                    nc.tensor.matmul(s2b[:kw, 512:512 + w],
                                     qkb[6 + hp][64:128, k0:k0 + kw],
                                     qkb[hp][64:128, q0:q0 + w],
                                     start=True, stop=True)
                    p2 = ppool.tile([128, 2, 342], BF16, name="p2")[:kw, :, :w]
                    nc.scalar.activation(
                        p2,
                        s2b[:kw].rearrange("p (b c) -> p b c", b=2)[:, :, :w],
                        AFT.Exp, bias=0.0, scale=SCALE)
                    nc.tensor.matmul(o_e, vsb[kt][:kw, 2 * hp, :], p2[:, 0, :],
                                     start=(kt == 0), stop=(kt == NT - 1))
                    nc.tensor.matmul(o_o, vsb[kt][:kw, 2 * hp + 1, :], p2[:, 1, :],
                                     start=(kt == 0), stop=(kt == NT - 1))
                nc.vector.tensor_copy(oraw_e[:, q0:q0 + w], o_e)
                nc.vector.tensor_copy(oraw_o[:, q0:q0 + w], o_o)

            # denominators (row 64) -> partitions 0-1 (the custom DVE
            # reciprocal only works at partition base 0) -> one batched
            # reciprocal -> DRAM-staged stride-0 partition broadcast.
            den = obuf.tile([2, NP], F32, name="den")
            nc.sync.dma_start(out=den[0:1, :], in_=oraw_e[64:65, :])
            nc.sync.dma_start(out=den[1:2, :], in_=oraw_o[64:65, :])
            nc.vector.reciprocal_approx_fast(den, den)
            dtmp = dstage.tile([2, NP], F32, name="dtmp")
            nc.sync.dma_start(out=dtmp[:, :], in_=den)
            bc_e = obuf.tile([64, NP], F32, name="bc_e")
            bc_o = obuf.tile([64, NP], F32, name="bc_o")
            nc.gpsimd.dma_start(out=bc_e, in_=_bcast_row(dtmp[0:1, :], 64))
            nc.gpsimd.dma_start(out=bc_o, in_=_bcast_row(dtmp[1:2, :], 64))
            nc.vector.tensor_mul(projrhs[hp][0:64, :], oraw_e[0:64, :], bc_e)
            onorm_o = obuf.tile([64, NP], F32R, name="onorm_o")
            nc.vector.tensor_mul(onorm_o, oraw_o[0:64, :], bc_o)
            # odd head lives on partitions 64-127 of the proj rhs tile;
            # DMA is the only engine that can shift partition ranges.
            nc.sync.dma_start(out=projrhs[hp][64:128, :], in_=onorm_o)

        # ---------------- output projection --------------------------------
        inner.close()
        with tc.tile_pool(name="wyb", bufs=1) as wyb, \
             tc.tile_pool(name="ybuf", bufs=3) as ybuf:
            wproj = [wyb.tile([128, C], F32R, name=f"wp{kk}") for kk in range(KT)]
            for kk in range(KT):
                nc.sync.dma_start(out=wproj[kk],
                                  in_=d["wprojt"][kk * 128:(kk + 1) * 128, :])
            for m in range(6):
                for (q0, w) in QSLICES:
                    ps = psA.tile([128, 512], F32, name="mmps")[:, :w]
                    for kk in range(KT):
                        nc.tensor.matmul(
                            ps,
                            wproj[kk][:, m * 128:(m + 1) * 128],
                            projrhs[kk][:, q0:q0 + w],
                            start=(kk == 0), stop=(kk == KT - 1),
                        )
                    yt = ybuf.tile([128, 512], F32, name="yt")[:, :w]
                    nc.scalar.activation(yt, ps, AFT.Identity,
                                         bias=projbt[:, m:m + 1], scale=1.0)
                    nc.sync.dma_start(out=d["out"][m * 128:(m + 1) * 128, q0:q0 + w],
                                      in_=yt)


_NC = None


def build_nc():
    global _NC
    if _NC is None:
        nc = bacc.Bacc("TRN2", target_bir_lowering=False, debug=False)
        d = {
            "xt": nc.dram_tensor("xt", [C, NP], BF16, kind="ExternalInput").ap(),
            "wqkvt": nc.dram_tensor("wqkvt", [C, 3 * C], BF16, kind="ExternalInput").ap(),
            "wprojt": nc.dram_tensor("wprojt", [C, C], F32R, kind="ExternalInput").ap(),
            "s2t": nc.dram_tensor("s2t", [128, 128], F32R, kind="ExternalInput").ap(),
            "cost2": nc.dram_tensor("cost2", [128, NP], F32, kind="ExternalInput").ap(),
            "sint2": nc.dram_tensor("sint2", [128, NP], F32, kind="ExternalInput").ap(),
            "qkvbt": nc.dram_tensor("qkvbt", [128, 12], F32, kind="ExternalInput").ap(),
            "projbt": nc.dram_tensor("projbt", [128, 6], F32, kind="ExternalInput").ap(),
            "vb": nc.dram_tensor("vb", [1, C], BF16, kind="ExternalInput").ap(),
            "ones1": nc.dram_tensor("ones1", [1, 128], BF16, kind="ExternalInput").ap(),
            "vones": nc.dram_tensor("vones", [128, 12, 1], BF16, kind="ExternalInput").ap(),
            "vzrow": nc.dram_tensor("vzrow", [1, 12, 65], BF16, kind="ExternalInput").ap(),
            "out": nc.dram_tensor("out", [C, NP], F32, kind="ExternalOutput").ap(),
        }
        with tile.TileContext(nc) as tc:
            _emit(nc, tc, d)
        nc.compile()
        _NC = nc
    return _NC


try:
    import ml_dtypes
    _bf16 = ml_dtypes.bfloat16
except ImportError:  # pragma: no cover
    _bf16 = np.float16


def _round_fp32r(a):
    """Round fp32 array to the fp32r grid (11-bit mantissa, RNE)."""
    u = np.ascontiguousarray(a, dtype=np.float32).view(np.uint32).copy()
    u += 0x7FF + ((u >> 12) & 1)
    u &= 0xFFFFF000
    return u.view(np.float32)


def make_in_maps(inputs):
    x = np.ascontiguousarray(np.asarray(inputs["x"], dtype=np.float32))
    sin = np.asarray(inputs["sin"], dtype=np.float32)
    cos = np.asarray(inputs["cos"], dtype=np.float32)
    qkv_w = np.asarray(inputs["qkv_w"], dtype=np.float32)
    qkv_b = np.asarray(inputs["qkv_b"], dtype=np.float32)
    proj_w = np.asarray(inputs["proj_w"], dtype=np.float32)
    proj_b = np.asarray(inputs["proj_b"], dtype=np.float32)

    # rotate_half as a signed permutation: rot = S64 @ q (per 64-dim head).
    s64 = np.zeros((64, 64), dtype=np.float32)
    for dd in range(32):
        s64[dd, dd + 32] = -1.0
        s64[dd + 32, dd] = 1.0
    s2t = np.zeros((128, 128), dtype=np.float32)
    s2t[:64, :64] = s64.T
    s2t[64:, 64:] = s64.T

    # [128, NP] sin/cos in [dim, token] layout, doubled for the head pair in
    # each 128-channel tile; col 0 (cls) and col 1025 (pad) get cos=1, sin=0.
    cost2 = np.ones((128, NP), dtype=np.float32)
    sint2 = np.zeros((128, NP), dtype=np.float32)
    cost2[:64, 1:N] = cos.T
    cost2[64:, 1:N] = cos.T
    sint2[:64, 1:N] = sin.T
    sint2[64:, 1:N] = sin.T

    shared = {
        "wqkvt": np.ascontiguousarray(qkv_w.T).astype(_bf16),
        "wprojt": _round_fp32r(proj_w.T),
        "s2t": s2t,
        "cost2": cost2,
        "sint2": sint2,
        "qkvbt": np.ascontiguousarray(qkv_b[:2 * C].reshape(12, 128).T),
        "projbt": np.ascontiguousarray(proj_b.reshape(6, 128).T),
        "vb": qkv_b[2 * C:].reshape(1, C).astype(_bf16),
        "ones1": np.ones((1, 128), dtype=_bf16),
        "vones": np.ones((128, 12, 1), dtype=_bf16),
        "vzrow": np.zeros((1, 12, 65), dtype=_bf16),
    }
    xp = np.zeros((C, NP), dtype=np.float32)
    maps = []
    for b in range(B):
        xp[:, :N] = x[b].T
        maps.append(dict(shared, xt=xp.astype(_bf16)))
    return maps


def kernel(**inputs) -> np.ndarray:
    nc = build_nc()
    in_maps = make_in_maps(inputs)
    res = run_bass_kernel_spmd(nc, in_maps, core_ids=list(range(B)))
    return np.stack([res.results[b]["out"][:, :N].T for b in range(B)]).astype(np.float32)


# revision 16
# speedup vs baseline: 2.0071x; 1.0122x over previous
"""Trainium2 Bass kernel for nn_Attention (B=8, N=1025, C=768, H=12 heads).

Sharding: pure data parallel — each of the 8 NeuronCores runs the full
attention block for one batch element; host slices/transposes inputs and
stacks outputs.  All device tensors are kept in "transposed" [channel, token]
layout so no on-device transposes are ever needed:

  qkvT[ch, tok] = wqkvT.T @ xT            (q,k; channel on partitions)
  RoPE via signed-permutation matmul (rotate_half = S @ q) + DVE elementwise
  v[tok, ch]    = xT.T @ wvT              (token on partitions, per-head
                                           65-col blocks + ones column)
  ST[k, q]      = kT.T @ qT               (per head pair, row-packed on the
                                           128x128 PE: even head rows 0-63,
                                           odd rows 64-127, one 2-bank psum
                                           tile so the pair issues adjacent
                                           and runs CONCURRENTLY)
  P = exp(ST/8)                           (single strided ScalarE op over
                                           both heads; no max subtraction —
                                           logits std ~2, max ~11, safe)
  O_aug[65, q]  = v_aug.T @ P             (PSUM-accumulated over k tiles;
                                           row 64 = softmax denominator via
                                           the ones column)
  normalize: 1/denom via batched custom-DVE reciprocal at partition base 0
  (the custom op ignores nonzero partition bases!), broadcast across the 64
  head dims with a DRAM-staged stride-0 DMA, odd head moved to partitions
  64-127 by SBUF-to-SBUF DMA; then yT = wprojT.T @ oT.

Precision: scores/AV/qkv matmuls in bf16 (fp32 PSUM accumulation), RoPE
arithmetic in fp32, output projection in float32r (TF32-like, 2 cyc/row).
Measured on TRN2: ~267 us/core NEFF exec, rel L2 err ~8.9e-3 vs the fp32
jax reference.  fp32r rules honored: even free dims (tokens padded
1025->1026, pad token's V row + ones column zeroed so softmax is exact),
f32r matmul inputs produced "rounded" (f32r-typed DMA loads with host
pre-rounding, or f32r-typed compute writes).

Emission order: V first, then per head pair (qkv -> RoPE -> attention), so
the ScalarE exp stream overlaps projection matmuls; output projection last.
PSUM budget exactly 8 banks: qkv/rope/v/proj accumulators (2) + score pair
tiles (2x2) + O accumulators (1+1).
"""

import numpy as np

import concourse.bass as bass
import concourse.bacc as bacc
import concourse.tile as tile
from concourse import mybir
from concourse.bass_utils import run_bass_kernel_spmd

B, N, C, H, HD = 8, 1025, 768, 12, 64
NP = 1026                # padded token count (fp32r needs even free dims)
SCALE = HD ** -0.5
KT = C // 128            # 6 contraction tiles over channels
NT = (NP + 127) // 128   # 9 token tiles (last holds 2 tokens: 1 real + 1 pad)
TAILW = NP - 128 * (NT - 1)
QSLICES = [(0, 342), (342, 342), (684, 342)]
F32 = mybir.dt.float32
F32R = mybir.dt.float32r
BF16 = mybir.dt.bfloat16
AFT = mybir.ActivationFunctionType
ALU = mybir.AluOpType


def _r(ap):
    return ap.bitcast(F32R)


def _bcast_row(row_ap, parts):
    """AP reading a [1, W] DRAM row replicated across `parts` partitions."""
    return bass.AP(
        tensor=row_ap.tensor,
        offset=row_ap.offset,
        ap=[[0, parts]] + list(row_ap.ap[1:]),
    )


def _emit(nc, tc, d):
    from contextlib import ExitStack

    with ExitStack() as ctx:
        const = ctx.enter_context(tc.tile_pool(name="const", bufs=1))
        s2t = const.tile([128, 128], F32R, name="s2t_sb")
        nc.sync.dma_start(out=s2t, in_=d["s2t"])
        cost2 = const.tile([128, NP], F32, name="cost2_sb")
        nc.sync.dma_start(out=cost2, in_=d["cost2"])
        sint2 = const.tile([128, NP], F32, name="sint2_sb")
        nc.sync.dma_start(out=sint2, in_=d["sint2"])
        qkvbt = const.tile([128, 12], F32, name="qkvbt_sb")
        nc.sync.dma_start(out=qkvbt, in_=d["qkvbt"])
        projbt = const.tile([128, 6], F32, name="projbt_sb")
        nc.sync.dma_start(out=projbt, in_=d["projbt"])
        vb = const.tile([1, C], BF16, name="vb_sb")
        nc.sync.dma_start(out=vb, in_=d["vb"])
        ones1 = const.tile([1, 128], BF16, name="ones1")
        nc.sync.dma_start(out=ones1, in_=d["ones1"])

        qk_pool = ctx.enter_context(tc.tile_pool(name="qkp", bufs=1))
        qkb = [qk_pool.tile([128, NP], BF16, name=f"qkb{m}") for m in range(12)]
        v_pool = ctx.enter_context(tc.tile_pool(name="vp", bufs=1))
        vsb = [v_pool.tile([128, 12, 65], BF16, name=f"v{t}") for t in range(NT)]

        att = ctx.enter_context(tc.tile_pool(name="att", bufs=1))
        projrhs = [att.tile([128, NP], F32R, name=f"prhs{hp}") for hp in range(6)]

        psA = ctx.enter_context(tc.tile_pool(name="psA", bufs=2, space="PSUM"))
        inner = ctx.enter_context(ExitStack())
        xw = inner.enter_context(tc.tile_pool(name="xw", bufs=1))
        wqkp = inner.enter_context(tc.tile_pool(name="wqkp", bufs=3))
        qkr_pool = inner.enter_context(tc.tile_pool(name="qkr", bufs=2))
        tmp1 = inner.enter_context(tc.tile_pool(name="tmp1", bufs=6))
        ppool = inner.enter_context(tc.tile_pool(name="ppool", bufs=4))
        obuf = inner.enter_context(tc.tile_pool(name="obuf", bufs=2))
        dstage = inner.enter_context(tc.tile_pool(name="dstage", bufs=3, space="DRAM"))
        # PSUM budget is exactly 8 banks:
        #   mm(2) + scores e/o (2+2) + O accumulators e/o (1+1) = 8
        psS = inner.enter_context(tc.tile_pool(name="psS", bufs=2, space="PSUM"))
        psO = inner.enter_context(tc.tile_pool(name="psO", bufs=1, space="PSUM"))

        # input DMAs: x and the V columns of wqkv first so the v matmuls (and
        # with them the first attention pairs) start as early as possible.
        # qk weight columns stream in per head pair; wproj loads at the end.
        xt = [xw.tile([128, NP], BF16, name=f"xt{kk}") for kk in range(KT)]
        wqv = [xw.tile([128, C], BF16, name=f"wqv{kk}") for kk in range(KT)]
        for kk in range(KT):
            nc.sync.dma_start(out=xt[kk], in_=d["xt"][kk * 128:(kk + 1) * 128, :])
            nc.sync.dma_start(out=wqv[kk], in_=d["wqkvt"][kk * 128:(kk + 1) * 128, 2 * C:])

        # ---- v in [tok, ch] layout, per-head 65-col blocks + ones column ---
        for t in range(NT):
            tw = 128 if t < NT - 1 else TAILW
            for vj, (v0, wv) in enumerate([(0, 512), (512, 256)]):
                ps = psA.tile([128, 512], F32, name="mmps")[:tw, :wv]
                nc.tensor.matmul(ps, ones1[:1, :tw], vb[:1, v0:v0 + wv],
                                 start=True, stop=False)
                for kk in range(KT):
                    nc.tensor.matmul(
                        ps,
                        xt[kk][:, t * 128:t * 128 + tw],
                        wqv[kk][:, v0:v0 + wv],
                        start=False, stop=(kk == KT - 1),
                    )
                nc.vector.tensor_copy(
                    vsb[t][:tw, vj * 8:vj * 8 + wv // 64, 0:64],
                    ps.rearrange("p (h dd) -> p h dd", dd=64),
                )
            # ones column for real tokens; the pad-token row of the tail tile
            # is fully zeroed (keeps denominators exact even w/ nonzero bias).
            if t < NT - 1:
                nc.sync.dma_start(out=vsb[t][:tw, :, 64:65], in_=d["vones"][:tw])
            else:
                nc.sync.dma_start(out=vsb[t][0:tw - 1, :, 64:65],
                                  in_=d["vones"][:tw - 1])
                nc.sync.dma_start(out=vsb[t][tw - 1:tw, :, :], in_=d["vzrow"])

        # ---- per head pair: qkv -> rope -> attention -----------------------
        for hp in range(6):
            for m in (hp, 6 + hp):
                wm = [wqkp.tile([128, 128], BF16, name=f"wqk{kk}") for kk in range(KT)]
                for kk in range(KT):
                    nc.sync.dma_start(
                        out=wm[kk],
                        in_=d["wqkvt"][kk * 128:(kk + 1) * 128, m * 128:(m + 1) * 128])
                qkr = qkr_pool.tile([128, NP], F32, name=f"qkr{'qk'[m >= 6]}")
                for (q0, w) in QSLICES:
                    ps = psA.tile([128, 512], F32, name="mmps")[:, :w]
                    for kk in range(KT):
                        nc.tensor.matmul(
                            ps,
                            wm[kk],
                            xt[kk][:, q0:q0 + w],
                            start=(kk == 0), stop=(kk == KT - 1),
                        )
                    # eviction + bias on DVE (keeps ScalarE free for exp)
                    nc.vector.tensor_scalar_add(
                        out=_r(qkr[:, q0:q0 + w]), in0=ps,
                        scalar1=qkvbt[:, m:m + 1],
                    )
                    # RoPE: rope = qk*cos + (S @ qk)*sin, written as bf16
                    rps = psA.tile([128, 512], F32, name="mmps")[:, :w]
                    nc.tensor.matmul(rps, s2t, _r(qkr[:, q0:q0 + w]),
                                     start=True, stop=True)
                    a_t = tmp1.tile([128, 342], F32, name="ropea")[:, :w]
                    nc.gpsimd.tensor_mul(a_t, qkr[:, q0:q0 + w], cost2[:, q0:q0 + w])
                    b_t = tmp1.tile([128, 342], F32, name="ropeb")[:, :w]
                    nc.vector.tensor_mul(b_t, rps, sint2[:, q0:q0 + w])
                    nc.vector.tensor_add(qkb[m][:, q0:q0 + w], a_t, b_t)

            oraw_e = obuf.tile([65, NP], F32, name="oraw_e")
            oraw_o = obuf.tile([65, NP], F32, name="oraw_o")
            for (q0, w) in QSLICES:
                o_e = psO.tile([65, 512], F32, name="o_e")[:, :w]
                o_o = psO.tile([65, 512], F32, name="o_o")[:, :w]
                for kt in range(NT):
                    kw = 128 if kt < NT - 1 else TAILW
                    k0 = kt * 128
                    # both heads' score tiles live in ONE 2-bank psum tile so
                    # the row-packed pair issues back-to-back (concurrent on
                    # the PE: even head rows 0-63, odd head rows 64-127) and
                    # a single strided exp consumes both.
                    s2b = psS.tile([128, 1024], F32, name="s2b")
                    nc.tensor.matmul(s2b[:kw, 0:w], qkb[6 + hp][0:64, k0:k0 + kw],
                                     qkb[hp][0:64, q0:q0 + w],
                                     start=True, stop=True)
                    nc.tensor.matmul(s2b[:kw, 512:512 + w],
                                     qkb[6 + hp][64:128, k0:k0 + kw],
                                     qkb[hp][64:128, q0:q0 + w],
                                     start=True, stop=True)
                    p2 = ppool.tile([128, 2, 342], BF16, name="p2")[:kw, :, :w]
                    nc.scalar.activation(
                        p2,
                        s2b[:kw].rearrange("p (b c) -> p b c", b=2)[:, :, :w],
                        AFT.Exp, bias=0.0, scale=SCALE)
                    nc.tensor.matmul(o_e, vsb[kt][:kw, 2 * hp, :], p2[:, 0, :],
                                     start=(kt == 0), stop=(kt == NT - 1))
                    nc.tensor.matmul(o_o, vsb[kt][:kw, 2 * hp + 1, :], p2[:, 1, :],
                                     start=(kt == 0), stop=(kt == NT - 1))
                nc.vector.tensor_copy(oraw_e[:, q0:q0 + w], o_e)
                nc.vector.tensor_copy(oraw_o[:, q0:q0 + w], o_o)

            # denominators (row 64) -> partitions 0-1 (the custom DVE
            # reciprocal only works at partition base 0) -> one batched
            # reciprocal -> DRAM-staged stride-0 partition broadcast.
            den = obuf.tile([2, NP], F32, name="den")
            nc.sync.dma_start(out=den[0:1, :], in_=oraw_e[64:65, :])
            nc.sync.dma_start(out=den[1:2, :], in_=oraw_o[64:65, :])
            nc.vector.reciprocal_approx_fast(den, den)
            dtmp = dstage.tile([2, NP], F32, name="dtmp")
            nc.sync.dma_start(out=dtmp[:, :], in_=den)
            bc_e = obuf.tile([64, NP], F32, name="bc_e")
            bc_o = obuf.tile([64, NP], F32, name="bc_o")
            nc.gpsimd.dma_start(out=bc_e, in_=_bcast_row(dtmp[0:1, :], 64))
            nc.gpsimd.dma_start(out=bc_o, in_=_bcast_row(dtmp[1:2, :], 64))
            nc.vector.tensor_mul(projrhs[hp][0:64, :], oraw_e[0:64, :], bc_e)
            onorm_o = obuf.tile([64, NP], F32R, name="onorm_o")
            nc.vector.tensor_mul(onorm_o, oraw_o[0:64, :], bc_o)
            # odd head lives on partitions 64-127 of the proj rhs tile;
            # DMA is the only engine that can shift partition ranges.
            nc.sync.dma_start(out=projrhs[hp][64:128, :], in_=onorm_o)

        # ---------------- output projection --------------------------------
        inner.close()
        with tc.tile_pool(name="wyb", bufs=1) as wyb, \
             tc.tile_pool(name="ybuf", bufs=3) as ybuf:
            wproj = [wyb.tile([128, C], F32R, name=f"wp{kk}") for kk in range(KT)]
            for kk in range(KT):
                nc.sync.dma_start(out=wproj[kk],
                                  in_=d["wprojt"][kk * 128:(kk + 1) * 128, :])
            for m in range(6):
                for (q0, w) in QSLICES:
                    ps = psA.tile([128, 512], F32, name="mmps")[:, :w]
                    for kk in range(KT):
                        nc.tensor.matmul(
                            ps,
                            wproj[kk][:, m * 128:(m + 1) * 128],
                            projrhs[kk][:, q0:q0 + w],
                            start=(kk == 0), stop=(kk == KT - 1),
                        )
                    yt = ybuf.tile([128, 512], F32, name="yt")[:, :w]
                    nc.scalar.activation(yt, ps, AFT.Identity,
                                         bias=projbt[:, m:m + 1], scale=1.0)
                    nc.sync.dma_start(out=d["out"][m * 128:(m + 1) * 128, q0:q0 + w],
                                      in_=yt)


_NC = None


def build_nc():
    global _NC
    if _NC is None:
        nc = bacc.Bacc("TRN2", target_bir_lowering=False, debug=False)
        d = {
            "xt": nc.dram_tensor("xt", [C, NP], BF16, kind="ExternalInput").ap(),
            "wqkvt": nc.dram_tensor("wqkvt", [C, 3 * C], BF16, kind="ExternalInput").ap(),
            "wprojt": nc.dram_tensor("wprojt", [C, C], F32R, kind="ExternalInput").ap(),
            "s2t": nc.dram_tensor("s2t", [128, 128], F32R, kind="ExternalInput").ap(),
            "cost2": nc.dram_tensor("cost2", [128, NP], F32, kind="ExternalInput").ap(),
            "sint2": nc.dram_tensor("sint2", [128, NP], F32, kind="ExternalInput").ap(),
            "qkvbt": nc.dram_tensor("qkvbt", [128, 12], F32, kind="ExternalInput").ap(),
            "projbt": nc.dram_tensor("projbt", [128, 6], F32, kind="ExternalInput").ap(),
            "vb": nc.dram_tensor("vb", [1, C], BF16, kind="ExternalInput").ap(),
            "ones1": nc.dram_tensor("ones1", [1, 128], BF16, kind="ExternalInput").ap(),
            "vones": nc.dram_tensor("vones", [128, 12, 1], BF16, kind="ExternalInput").ap(),
            "vzrow": nc.dram_tensor("vzrow", [1, 12, 65], BF16, kind="ExternalInput").ap(),
            "out": nc.dram_tensor("out", [C, NP], F32, kind="ExternalOutput").ap(),
        }
        with tile.TileContext(nc) as tc:
            _emit(nc, tc, d)
        nc.compile()
        _NC = nc
    return _NC


try:
    import ml_dtypes
    _bf16 = ml_dtypes.bfloat16
except ImportError:  # pragma: no cover
    _bf16 = np.float16


def _round_fp32r(a):
    """Round fp32 array to the fp32r grid (11-bit mantissa, RNE)."""
    u = np.ascontiguousarray(a, dtype=np.float32).view(np.uint32).copy()
    u += 0x7FF + ((u >> 12) & 1)
    u &= 0xFFFFF000
    return u.view(np.float32)


def make_in_maps(inputs):
    x = np.ascontiguousarray(np.asarray(inputs["x"], dtype=np.float32))
    sin = np.asarray(inputs["sin"], dtype=np.float32)
    cos = np.asarray(inputs["cos"], dtype=np.float32)
    qkv_w = np.asarray(inputs["qkv_w"], dtype=np.float32)
    qkv_b = np.asarray(inputs["qkv_b"], dtype=np.float32)
    proj_w = np.asarray(inputs["proj_w"], dtype=np.float32)
    proj_b = np.asarray(inputs["proj_b"], dtype=np.float32)

    # rotate_half as a signed permutation: rot = S64 @ q (per 64-dim head).
    s64 = np.zeros((64, 64), dtype=np.float32)
    for dd in range(32):
        s64[dd, dd + 32] = -1.0
        s64[dd + 32, dd] = 1.0
    s2t = np.zeros((128, 128), dtype=np.float32)
    s2t[:64, :64] = s64.T
    s2t[64:, 64:] = s64.T

    # [128, NP] sin/cos in [dim, token] layout, doubled for the head pair in
    # each 128-channel tile; col 0 (cls) and col 1025 (pad) get cos=1, sin=0.
    cost2 = np.ones((128, NP), dtype=np.float32)
    sint2 = np.zeros((128, NP), dtype=np.float32)
    cost2[:64, 1:N] = cos.T
    cost2[64:, 1:N] = cos.T
    sint2[:64, 1:N] = sin.T
    sint2[64:, 1:N] = sin.T

    shared = {
        "wqkvt": np.ascontiguousarray(qkv_w.T).astype(_bf16),
        "wprojt": _round_fp32r(proj_w.T),
        "s2t": s2t,
        "cost2": cost2,
        "sint2": sint2,
        "qkvbt": np.ascontiguousarray(qkv_b[:2 * C].reshape(12, 128).T),
        "projbt": np.ascontiguousarray(proj_b.reshape(6, 128).T),
        "vb": qkv_b[2 * C:].reshape(1, C).astype(_bf16),
        "ones1": np.ones((1, 128), dtype=_bf16),
        "vones": np.ones((128, 12, 1), dtype=_bf16),
        "vzrow": np.zeros((1, 12, 65), dtype=_bf16),
    }
    xp = np.zeros((C, NP), dtype=np.float32)
    maps = []
    for b in range(B):
        xp[:, :N] = x[b].T
        maps.append(dict(shared, xt=xp.astype(_bf16)))
    return maps


def kernel(**inputs) -> np.ndarray:
    nc = build_nc()
    in_maps = make_in_maps(inputs)
    res = run_bass_kernel_spmd(nc, in_maps, core_ids=list(range(B)))
    return np.stack([res.results[b]["out"][:, :N].T for b in range(B)]).astype(np.float32)
